# revision 32
# baseline (speedup 1.0000x reference)
"""Trainium2 Bass kernel for the AnaphoricityScorer (coref pair scorer).

Reference computation (per mention row m, antecedent slot j):
    b = all_mentions[idx[m, j]]                       # gather, 1024-dim
    pair = [a_m, b, a_m * b, pw[m, j]]                # 3136-dim
    h = leaky_relu(pair @ W1 + b1)                    # 1024-dim
    score[m, j] = rough[m, j] + h @ W_out + b_out
    out[m] = [EPS, score[m, 0..49]]                   # [1024, 51]

Strategy (8 NeuronCores, data-parallel over the 1024-mention batch):
  * 128 mentions per core; all_mentions + FFNN weights replicated.
  * Decompose W1 by pair-feature block: W1 = [W1_a; W1_b; W1_ab; W1_pw].
      - a-term:  A1^T = (W1_a^T a^T) computed once per core (tiny).
      - b-term:  BM = all_mentions @ W1_b precomputed SHARDED across the 8
        cores (1024 rows each) + AllGather, then per-pair rows of BM are
        *gathered* instead of re-multiplied (saves 13.4 GFLOP/core).
      - ab-term: full matmul (a (x) b)^T @ ... -> irreducible 13.4 GFLOP/core,
        run in fp8e4m3 with MatmulPerfMode.DoubleRow (2 K-chunks of 128 per
        instruction, ~2x stream rate; measured 2419 cyc vs bf16 4096 per
        K=1024 x N=512 group). W1 blocks carry a x64 scale (keeps e4m3 out of
        subnormals); leaky-relu is positively homogeneous, so /64 baked into
        W_out cancels it. rel err 1.0e-2 vs 1.8e-3 in bf16 (gate 2e-2).
      - pw-term: K=64 bf16 matmul.
  * dma_gather(transpose=True) both gathers antecedent rows AND delivers them
    feature-major ([emb%128, emb//128, paircol]) - exactly the moving-operand
    layout the matmul needs. Pair columns are ordered g = j*128 + m so each
    128-column group shares one antecedent slot j across all 128 mentions.
  * Per 512-column block and h-chunk, the PE stream is [4x ab DoubleRow + pw
    + identity-matmul(ba)]: the b-term enters PSUM via a PE identity matmul
    whose rhs ba = bmT + A1^T is precomputed on the DVE one block AHEAD
    (bmT gather + DVE add prefetched), and layer-2 (N=1 matmuls vs W_out)
    is DEFERRED one block - so the in-order PE queue never waits on a
    same-block DVE/scalar/gather product (DVE adds into PSUM measured
    3.1us/block exposed in the fused form). leaky-relu on the ScalarEngine
    (Lrelu) -> bf16 h^T consumed by next block's layer-2.
  * bT gathers prefetched 4 blocks ahead (the first bmT gather stalls the
    FIFO gather queue until the AllGather lands).
  * ALL inputs are packed into ONE bf16 DRAM blob per core (f32/i16/fp8
    sections bitcast) - the axon per-execute dispatch cost scales with the
    input tensor count (~60-120us per input x device), so 17 inputs -> 1
    (measured -870us/exec head-to-head).
"""

import numpy as np
import ml_dtypes

CORES = 8
BATCH = 1024
B_LOC = BATCH // CORES          # 128 mentions per core
N_ANTS = 50
EMB = 1024
PW = 64
HID = 1024
N_MENT = 8192
G = B_LOC * N_ANTS              # 6400 pair columns per core
EPS = 1e-7

COLS_PER_BLK = 512              # pair columns per block (4 antecedent slots)
N_BLK = (G + COLS_PER_BLK - 1) // COLS_PER_BLK   # 13 (12 full + 1 half)

SIM_SAFE = False                # set True before get_nc() for CoreSim runs
import os as _os
DBG_NO_COLL = _os.environ.get("KDBG_NO_COLL", "0") == "1"   # skip AllGather (wrong results)
DBG_NBLK = int(_os.environ.get("KDBG_NBLK", "0"))           # limit main-loop blocks
DBG_NO_LRELU = _os.environ.get("KDBG_NO_LRELU", "0") == "1"
DBG_REPEAT = int(_os.environ.get("KDBG_REPEAT", "1"))  # repeat main loop (timing)
DBG_AG_REPEAT = int(_os.environ.get("KDBG_AG_REPEAT", "1"))  # repeat AllGather
DBG_BM_REPEAT = int(_os.environ.get("KDBG_BM_REPEAT", "1"))  # repeat BM compute
DBG_NO_BT = _os.environ.get("KDBG_NO_BT", "0") == "1"    # skip bT gather (wrong)
DBG_NO_BMT = _os.environ.get("KDBG_NO_BMT", "0") == "1"  # skip bmT gather+add (wrong)
DBG_NO_BA = _os.environ.get("KDBG_NO_BA", "0") == "1"    # keep gather, skip adds (wrong)
FP8 = _os.environ.get("KDBG_FP8", "1") == "1"  # ab-term matmuls in fp8e4m3
FP8_SCALE = 64.0  # keeps 64*W1ab out of the e4m3 subnormal range; lrelu is
# positively homogeneous so baking x64 into W1 and /64 into W_out cancels

_BF16 = ml_dtypes.bfloat16

_cache = {}


def _prod(s):
    p = 1
    for x in s:
        p *= x
    return p


def _align(x, a=256):
    return (x + a - 1) // a * a


# ---- blob layout: (name, shape, elem-kind). Kinds: bf16/i16 (1 blob elem),
# f32 (2 blob elems), fp8 (1/2 blob elem). Offsets in bf16 elems, 256-aligned.
def _sections():
    return [
        ("am",    [N_MENT, EMB],        "bf16"),
        ("ident", [128, 128],           "bf16"),
        ("w1a",   [128, 8, HID],        "bf16"),
        ("w1b",   [128, 8, HID],        "bf16"),
        ("w1ab",  [128, 8, HID],        "fp8" if FP8 else "bf16"),
        ("w1pw",  [PW, HID],            "bf16"),
        ("aT",    [128, 8, B_LOC],      "bf16"),
        ("aTr",   [128, 8, 512],        "bf16"),
        ("pwT",   [PW, N_ANTS, B_LOC],  "bf16"),
        ("wout",  [128, 8],             "bf16"),
        ("rough", [B_LOC, N_ANTS],      "f32"),
        ("b1T",   [128, 8],             "f32"),
        ("boutr", [128, 1],             "f32"),
        ("idx",   [128, G // 16],       "i16"),
        ("idxid", [128, B_LOC * 8 // 16], "i16"),
    ]


def _elems(shape, kind):
    n = _prod(shape)
    if kind == "f32":
        return n * 2
    if kind == "fp8":
        assert n % 2 == 0
        return n // 2
    return n


def _blob_offsets():
    offs, off = {}, 0
    for name, shape, kind in _sections():
        offs[name] = off
        off = _align(off + _elems(shape, kind))
    return offs, off


def _build_nc():
    from contextlib import ExitStack

    import concourse.bacc as bacc
    import concourse.mybir as mybir
    import concourse.tile as tile

    f32 = mybir.dt.float32
    bf16 = mybir.dt.bfloat16
    i16 = mybir.dt.int16
    fp8 = mybir.dt.float8e4
    AF = mybir.ActivationFunctionType
    ALU = mybir.AluOpType

    nc = bacc.Bacc("TRN2", target_bir_lowering=False, debug=False,
                   num_devices=CORES)

    offs, blob_len = _blob_offsets()
    blob_d = nc.dram_tensor("blob", [blob_len], bf16, kind="ExternalInput")
    out_d = nc.dram_tensor("out", [B_LOC, 1 + N_ANTS], f32, kind="ExternalOutput")

    _dt = {"bf16": bf16, "f32": f32, "i16": i16, "fp8": fp8}
    _kinds = {name: kind for name, _, kind in _sections()}
    _shapes = {name: shape for name, shape, _ in _sections()}

    def view(name):
        shape, kind = _shapes[name], _kinds[name]
        ap = blob_d[offs[name]:offs[name] + _elems(shape, kind)]
        if kind != "bf16":
            ap = ap.bitcast(_dt[kind])
        if len(shape) == 1:
            return ap
        dims = "abcde"[:len(shape)]
        kw = {d: s for d, s in zip(dims, shape)}
        return ap.rearrange(f"({' '.join(dims)}) -> {' '.join(dims)}", **kw)

    with tile.TileContext(nc) as tc, ExitStack() as ctx:
        const = ctx.enter_context(tc.tile_pool(name="const", bufs=1))
        dram = ctx.enter_context(tc.tile_pool(name="dram", bufs=1, space="DRAM"))
        gpool = ctx.enter_context(tc.tile_pool(name="gather", bufs=2))
        wpool = ctx.enter_context(tc.tile_pool(name="work", bufs=2))
        hpool = ctx.enter_context(tc.tile_pool(name="hsb", bufs=2))
        plp_cm = tc.tile_pool(name="prolog", bufs=1)
        plp = plp_cm.__enter__()
        pp = ctx.enter_context(tc.tile_pool(name="psum", bufs=6, space="PSUM"))
        fp = ctx.enter_context(tc.tile_pool(name="fpsum", bufs=2, space="PSUM"))

        def load(name, dt, eng=None, pool=None):
            shape = _shapes[name]
            t = (pool or const).tile(shape, dt, tag=name)
            (eng or nc.sync).dma_start(t[:], view(name))
            return t

        # load order matters: idx tiles + small consts first so block-0
        # gathers and matmuls can start while the big weights stream in.
        # Big weights go on the scalar HWDGE queue, small/critical on sync.
        idx_sb = load("idx", i16)
        idxid_sb = load("idxid", i16)
        ident = load("ident", bf16)
        w1b = load("w1b", bf16, pool=plp)
        aT = load("aT", bf16, eng=nc.scalar)
        aTr = load("aTr", bf16, eng=nc.scalar)
        wout = load("wout", bf16, eng=nc.scalar)
        b1T = load("b1T", f32, eng=nc.scalar)
        bout = load("boutr", f32, eng=nc.scalar)
        rough = load("rough", f32, eng=nc.scalar)
        w1ab = load("w1ab", fp8 if FP8 else bf16, eng=nc.scalar)
        pwT = load("pwT", bf16, eng=nc.scalar)
        w1pw = load("w1pw", bf16, eng=nc.scalar)
        w1a = load("w1a", bf16, eng=nc.scalar, pool=plp)

        am_ap = view("am")

        # ---- BM = all_mentions @ W1_b for my 1024-row shard, then AllGather.
        # amT: my shard of all_mentions, feature-major, via identity gather.
        # Split into 512-idx halves: one gather's descriptors must fit the
        # SWDGE ring carveout (1024-idx transpose gathers fault on HW).
        amT_h = [plp.tile([128, 8, 256], bf16, tag=f"amT{h}", name=f"amT{h}")
                 for h in range(4)]

        bm_shard = dram.tile([B_LOC * 8, HID], bf16)

        for r in range(DBG_BM_REPEAT):
            for h in range(4):
                nc.gpsimd.dma_gather(amT_h[h][:], am_ap,
                                     idxid_sb[:, h * 16:(h + 1) * 16],
                                     256, 256, EMB, transpose=True)

            bm_all = plp.tile([128, 8, HID], bf16, tag="bm_all")
            for mi in range(8):
                for nh in range(2):
                    ps = pp.tile([128, 512], f32, tag="hp")
                    amT = amT_h[mi // 2]
                    mo = (mi % 2) * 128
                    for ce in range(8):
                        nc.tensor.matmul(
                            ps[:],
                            amT[:, ce, mo:mo + 128],
                            w1b[:, ce, nh * 512:(nh + 1) * 512],
                            start=(ce == 0), stop=(ce == 7))
                    nc.scalar.activation(bm_all[:, mi, nh * 512:(nh + 1) * 512],
                                         ps[:], AF.Copy)
                nc.sync.dma_start(bm_shard[mi * 128:(mi + 1) * 128, :],
                                  bm_all[:, mi, :])

        for r in range(DBG_AG_REPEAT):
            bm_full = dram.tile([N_MENT, HID], bf16, addr_space="Shared",
                                name=f"bm_full{r}")
            if not DBG_NO_COLL:
                nc.gpsimd.collective_compute(
                    "AllGather", ALU.bypass,
                    replica_groups=[list(range(CORES))],
                    ins=[bm_shard.opt()],
                    outs=[bm_full.opt()],
                )
            else:
                nc.sync.dma_start(bm_full[0:1024, :], bm_shard[:])

        # ---- A1^T = (a @ W1_a + b1)^T : [h%128, h//128, m], bf16,
        # replicated x4 along cols so the per-block add is one N=512 matmul.
        a1Tr = const.tile([128, 8, 512], bf16)
        for ch in range(8):
            ps = pp.tile([128, B_LOC], f32, tag="hp")
            for ce in range(8):
                nc.tensor.matmul(ps[:], w1a[:, ce, ch * 128:(ch + 1) * 128],
                                 aT[:, ce, :], start=(ce == 0), stop=(ce == 7))
            for j in range(4):
                nc.vector.tensor_scalar_add(a1Tr[:, ch, j * 128:(j + 1) * 128],
                                            ps[:], b1T[:, ch:ch + 1])

        plp_cm.__exit__(None, None, None)

        # ---- main loop over pair-column blocks
        scores = const.tile([B_LOC, 1 + N_ANTS], f32)
        nc.vector.memset(scores[:, 0:1], EPS)

        nblk = N_BLK if DBG_NBLK == 0 else max(DBG_NBLK, 0)
        PREF = 4   # bT gathers issued this many blocks ahead of bmT (the
        # bmT gather stalls the FIFO gather queue until the AllGather lands)
        blocks = [b for _ in range(DBG_REPEAT) for b in range(nblk)]

        def block_geom(B):
            nj = min(4, N_ANTS - B * 4)
            return nj, nj * 128, B * (COLS_PER_BLK // 16)

        def issue_bT(B):
            nj, NCOL, i0 = block_geom(B)
            t = gpool.tile([128, 8, NCOL], bf16, tag="bT", name=f"bT{B}", bufs=4)
            if DBG_NO_BT:
                # timing variant: same bytes via plain contiguous HWDGE DMA
                st = B * 128 * 8 * NCOL
                nc.sync.dma_start(t[:], blob_d[st:st + 128 * 8 * NCOL]
                                  .rearrange("(p c n) -> p c n", p=128, c=8))
            else:
                nc.gpsimd.dma_gather(t[:], am_ap,
                                     idx_sb[:, i0:i0 + NCOL // 16],
                                     NCOL, NCOL, EMB, transpose=True)
            return t

        bT_q = {}
        for i, B in enumerate(blocks[:PREF]):
            bT_q[i] = issue_bT(B)

        # bmT gather -> ba = bmT + A1^T and abT = bT * aT are produced one
        # block AHEAD of PE consumption, so the PE's in-order queue never
        # waits on a same-block DVE/gather product.
        ab_q = {}

        def issue_ab(bi2):
            B2 = blocks[bi2]
            nj2, NCOL2, i02 = block_geom(B2)
            abT = wpool.tile([128, 8, NCOL2], fp8 if FP8 else bf16, tag="abT")
            nc.vector.tensor_mul(abT[:], bT_q[bi2][:], aTr[:, :, 0:NCOL2])
            ba = None
            if not (DBG_NO_BMT or DBG_NO_BA):
                bmT = gpool.tile([128, 8, NCOL2], bf16, tag="bmT")
                if not DBG_NO_BMT:
                    nc.gpsimd.dma_gather(bmT[:], bm_full[:],
                                         idx_sb[:, i02:i02 + NCOL2 // 16],
                                         NCOL2, NCOL2, HID, transpose=True)
                ba = wpool.tile([128, 8, NCOL2], bf16, tag="ba")
                nc.vector.tensor_add(ba[:], bmT[:], a1Tr[:, :, 0:NCOL2])
            ab_q[bi2] = (abT, ba)

        issue_ab(0)

        # Per block: PE runs [4x ab DoubleRow + pw + identity-add(ba)] per
        # h-chunk; the b-term lands via the identity matmul (rhs = ba =
        # bmT + A1^T, precomputed on DVE with no dependence on this block's
        # psum), and layer-2 matmuls are DEFERRED one block so the in-order
        # PE queue never waits on this block's lrelu output.
        prev = None  # (hT, ffps, nj, j0) of previous block

        def issue_l2(pv, ch):
            hT_p, ffps_p, nj_p, _ = pv
            for j in range(nj_p):
                nc.tensor.matmul(ffps_p[:, j:j + 1],
                                 hT_p[:, ch, j * 128:(j + 1) * 128],
                                 wout[:, ch:ch + 1],
                                 start=(ch == 0 and j == 0),
                                 stop=(ch == 7 and j == nj_p - 1))

        def finish_scores(pv):
            _, ffps_p, nj_p, j0_p = pv
            # scores[:, 1+j0 : 1+j0+nj] = ffnn + b_out + rough
            nc.vector.scalar_tensor_tensor(scores[:, 1 + j0_p:1 + j0_p + nj_p],
                                           ffps_p[:], bout[:, 0:1],
                                           rough[:, j0_p:j0_p + nj_p],
                                           op0=ALU.add, op1=ALU.add)

        for bi, B in enumerate(blocks):
            nj, NCOL, i0 = block_geom(B)
            j0 = B * 4

            if bi + PREF < len(blocks):
                bT_q[bi + PREF] = issue_bT(blocks[bi + PREF])
            abT, ba = ab_q.pop(bi)
            bT_q.pop(bi)
            if bi + 1 < len(blocks):
                issue_ab(bi + 1)
            use_ba = ba is not None

            ffps = fp.tile([128, nj], f32, tag="ff")
            hT = hpool.tile([128, 8, NCOL], bf16, tag="hT")
            for ch in range(8):
                hp = pp.tile([128, NCOL], f32, tag="hp")
                if FP8:
                    # DoubleRow: 2 K-chunks of 128 per instruction, 2 rows
                    # per PE cell - halves both stream cycles and instr count
                    for cp in range(4):
                        nc.tensor.matmul(
                            hp[:],
                            w1ab[:, 2 * cp:2 * cp + 2, ch * 128:(ch + 1) * 128],
                            abT[:, 2 * cp:2 * cp + 2, :],
                            start=(cp == 0), stop=False,
                            perf_mode=mybir.MatmulPerfMode.DoubleRow)
                else:
                    for ce in range(8):
                        nc.tensor.matmul(hp[:],
                                         w1ab[:, ce, ch * 128:(ch + 1) * 128],
                                         abT[:, ce, :], start=(ce == 0), stop=False)
                nc.tensor.matmul(hp[:], w1pw[:, ch * 128:(ch + 1) * 128],
                                 pwT[:, j0:j0 + nj, :], start=False,
                                 stop=not use_ba)
                if use_ba:
                    nc.tensor.matmul(hp[:], ident[:],
                                     ba[:, ch, :], start=False, stop=True)
                if prev is not None:
                    issue_l2(prev, ch)

                # leaky_relu(x) = max(x, 0.01*x)
                if SIM_SAFE or DBG_NO_LRELU:  # CoreSim doesn't implement Lrelu
                    lt = hpool.tile([128, NCOL], f32, tag="lt")
                    nc.vector.tensor_scalar_mul(lt[:], hp[:], 0.01)
                    nc.vector.tensor_max(hT[:, ch, :], hp[:], lt[:])
                else:
                    nc.scalar.activation(hT[:, ch, :], hp[:], AF.Lrelu, alpha=0.01)

            if prev is not None:
                finish_scores(prev)
            prev = (hT, ffps, nj, j0)

        # epilogue: layer-2 + scores for the final block
        for ch in range(8):
            issue_l2(prev, ch)
        finish_scores(prev)

        nc.sync.dma_start(out_d[:], scores[:])

    nc.compile()
    return nc


def _shard(inputs):
    am = np.asarray(inputs["all_mentions"], np.float32)
    mb_ = np.asarray(inputs["mentions_batch"], np.float32)
    pw = np.asarray(inputs["pw_batch"], np.float32)
    rough = np.asarray(inputs["top_rough_scores_batch"], np.float32)
    W1 = np.asarray(inputs["W1"], np.float32)
    b1 = np.asarray(inputs["b1"], np.float32)
    Wout = np.asarray(inputs["W_out"], np.float32)
    bout = np.asarray(inputs["b_out"], np.float32)
    idx = np.asarray(inputs["top_indices_batch"])

    am_bf = am.astype(_BF16)
    S = FP8_SCALE if FP8 else 1.0

    def wblock(rows, dt=_BF16):  # [1024, 1024] -> [128, 8, 1024] (e%128, e//128, h)
        return np.ascontiguousarray(
            rows.reshape(8, 128, HID).transpose(1, 0, 2)).astype(dt)

    w1a = wblock(S * W1[0:1024])
    w1b = wblock(S * W1[1024:2048])
    w1ab = wblock(S * W1[2048:3072],
                  ml_dtypes.float8_e4m3fn if FP8 else _BF16)
    w1pw = np.ascontiguousarray(S * W1[3072:3136]).astype(_BF16)      # [64, 1024]
    wout = np.ascontiguousarray(
        Wout[:, 0].reshape(8, 128).T / S).astype(_BF16)
    b1T = np.ascontiguousarray(S * b1.reshape(8, 128).T).astype(np.float32)
    boutr = np.full((128, 1), bout[0], np.float32)

    def wrap_idx(flat, pad_cols):
        # [16, n/16] wrapped block, replicated across all 8 GPSIMD-core
        # partition groups (the odd Q7 core reads partitions 16-31).
        return np.tile(flat.reshape(pad_cols, 16).T, (8, 1))

    def as_u16(a):
        a = np.ascontiguousarray(a)
        if a.dtype.itemsize == 1:
            a = a.reshape(-1)
            assert a.size % 2 == 0
            return a.view(np.uint16)
        return a.view(np.uint16).reshape(-1)

    in_maps = []
    for c in range(CORES):
        sl = slice(c * B_LOC, (c + 1) * B_LOC)
        aT = np.ascontiguousarray(
            mb_[sl].T.reshape(8, 128, B_LOC).transpose(1, 0, 2)).astype(_BF16)
        aTr = np.ascontiguousarray(np.tile(aT, (1, 1, 4)))
        pwT = np.ascontiguousarray(pw[sl].transpose(2, 1, 0)).astype(_BF16)
        idx_flat = np.ascontiguousarray(idx[sl].astype(np.int16).T).reshape(G)
        idxid_flat = np.arange(c * B_LOC * 8, (c + 1) * B_LOC * 8, dtype=np.int16)
        sections = {
            "am": am_bf,
            "ident": np.eye(128, dtype=_BF16),
            "aT": aT,
            "aTr": aTr,
            "pwT": pwT,
            "rough": np.ascontiguousarray(rough[sl]),
            "w1a": w1a, "w1b": w1b, "w1ab": w1ab, "w1pw": w1pw,
            "wout": wout, "b1T": b1T, "boutr": boutr,
            "idx": wrap_idx(idx_flat, G // 16),
            "idxid": wrap_idx(idxid_flat, B_LOC * 8 // 16),
        }
        offs, blob_len = _blob_offsets()
        blob = np.zeros(blob_len, np.uint16)
        for name, shape, kind in _sections():
            u = as_u16(sections[name])
            blob[offs[name]:offs[name] + u.size] = u
        in_maps.append({"blob": blob.view(_BF16)})
    return in_maps


def get_nc():
    if "nc" not in _cache:
        _cache["nc"] = _build_nc()
    return _cache["nc"]


def kernel(**inputs):
    import time

    from concourse.bass_utils import run_bass_kernel_spmd

    nc = get_nc()
    in_maps = _shard(inputs)
    try:
        res = run_bass_kernel_spmd(nc, in_maps, core_ids=list(range(CORES)))
    except Exception:  # transient axon/NRT wedge: one retry
        time.sleep(20)
        res = run_bass_kernel_spmd(nc, in_maps, core_ids=list(range(CORES)))
    outs = [r["out"] for r in res.results]
    return np.concatenate(outs, axis=0)


# revision 43
# speedup vs baseline: 1.2655x; 1.2655x over previous
"""Trainium2 Bass kernel for the AnaphoricityScorer (coref pair scorer).

Reference computation (per mention row m, antecedent slot j):
    b = all_mentions[idx[m, j]]                       # gather, 1024-dim
    pair = [a_m, b, a_m * b, pw[m, j]]                # 3136-dim
    h = leaky_relu(pair @ W1 + b1)                    # 1024-dim
    score[m, j] = rough[m, j] + h @ W_out + b_out
    out[m] = [EPS, score[m, 0..49]]                   # [1024, 51]

Strategy (8 NeuronCores, data-parallel over the 1024-mention batch):
  * 128 mentions per core; all_mentions + FFNN weights replicated.
  * Decompose W1 by pair-feature block: W1 = [W1_a; W1_b; W1_ab; W1_pw].
      - a-term:  A1^T = (W1_a^T a^T) computed once per core (tiny).
      - b-term:  BM = all_mentions @ W1_b precomputed SHARDED across the 8
        cores (1024 rows each) + AllGather, then per-pair rows of BM are
        *gathered* instead of re-multiplied (saves 13.4 GFLOP/core).
      - ab-term: full matmul (a (x) b)^T @ ... -> irreducible 13.4 GFLOP/core,
        run in fp8e4m3 with MatmulPerfMode.DoubleRow (2 K-chunks of 128 per
        instruction, ~2x stream rate; measured 2419 cyc vs bf16 4096 per
        K=1024 x N=512 group). W1 blocks carry a x64 scale (keeps e4m3 out of
        subnormals); leaky-relu is positively homogeneous, so /64 baked into
        W_out cancels it. rel err 1.0e-2 vs 1.8e-3 in bf16 (gate 2e-2).
      - pw-term: K=64 bf16 matmul.
  * dma_gather(transpose=True) both gathers antecedent rows AND delivers them
    feature-major ([emb%128, emb//128, paircol]) - exactly the moving-operand
    layout the matmul needs. Pair columns are ordered g = j*128 + m so each
    128-column group shares one antecedent slot j across all 128 mentions.
  * Per 512-column block and h-chunk, the PE stream is [4x ab DoubleRow + pw
    + identity-matmul(ba)]: the b-term enters PSUM via a PE identity matmul
    whose rhs ba = bmT + A1^T is precomputed on the DVE one block AHEAD
    (bmT gather + DVE add prefetched), and layer-2 (N=1 matmuls vs W_out)
    is DEFERRED one block - so the in-order PE queue never waits on a
    same-block DVE/scalar/gather product (DVE adds into PSUM measured
    3.1us/block exposed in the fused form). leaky-relu on the ScalarEngine
    (Lrelu) -> bf16 h^T consumed by next block's layer-2.
  * bT gathers prefetched 4 blocks ahead (the first bmT gather stalls the
    FIFO gather queue until the AllGather lands).
  * ALL inputs are packed into ONE bf16 DRAM blob per core (f32/i16/fp8
    sections bitcast) - the axon per-execute dispatch cost scales with the
    input tensor count (~60-120us per input x device), so 17 inputs -> 1
    (measured -870us/exec head-to-head).
"""

import numpy as np
import ml_dtypes

CORES = 8
BATCH = 1024
B_LOC = BATCH // CORES          # 128 mentions per core
N_ANTS = 50
EMB = 1024
PW = 64
HID = 1024
N_MENT = 8192
G = B_LOC * N_ANTS              # 6400 pair columns per core
EPS = 1e-7

COLS_PER_BLK = 512              # pair columns per block (4 antecedent slots)
N_BLK = (G + COLS_PER_BLK - 1) // COLS_PER_BLK   # 13 (12 full + 1 half)

SIM_SAFE = False                # set True before get_nc() for CoreSim runs
import os as _os
DBG_NO_COLL = _os.environ.get("KDBG_NO_COLL", "0") == "1"   # skip AllGather (wrong results)
DBG_NBLK = int(_os.environ.get("KDBG_NBLK", "0"))           # limit main-loop blocks
DBG_NO_LRELU = _os.environ.get("KDBG_NO_LRELU", "0") == "1"
DBG_REPEAT = int(_os.environ.get("KDBG_REPEAT", "1"))  # repeat main loop (timing)
DBG_AG_REPEAT = int(_os.environ.get("KDBG_AG_REPEAT", "1"))  # repeat AllGather
DBG_BM_REPEAT = int(_os.environ.get("KDBG_BM_REPEAT", "1"))  # repeat BM compute
DBG_NO_BT = _os.environ.get("KDBG_NO_BT", "0") == "1"    # skip bT gather (wrong)
DBG_NO_BMT = _os.environ.get("KDBG_NO_BMT", "0") == "1"  # skip bmT gather+add (wrong)
DBG_NO_BA = _os.environ.get("KDBG_NO_BA", "0") == "1"    # keep gather, skip adds (wrong)
FP8 = _os.environ.get("KDBG_FP8", "1") == "1"  # ab-term matmuls in fp8e4m3
# Blocks whose ab+pw psum is staged to SBUF during the AllGather; their
# b-term is combined later on DVE/scalar while the normal loop runs on PE.
STAGE = int(_os.environ.get("KDBG_STAGE", "3"))
FP8_SCALE = 64.0  # keeps 64*W1ab out of the e4m3 subnormal range; lrelu is
# positively homogeneous so baking x64 into W1 and /64 into W_out cancels

_BF16 = ml_dtypes.bfloat16

_cache = {}


def _prod(s):
    p = 1
    for x in s:
        p *= x
    return p


def _align(x, a=256):
    return (x + a - 1) // a * a


# ---- blob layout: (name, shape, elem-kind). Kinds: bf16/i16 (1 blob elem),
# f32 (2 blob elems), fp8 (1/2 blob elem). Offsets in bf16 elems, 256-aligned.
def _sections():
    return [
        ("am",    [N_MENT, EMB],        "bf16"),
        ("ident", [128, 128],           "bf16"),
        ("w1a",   [128, 8, HID],        "bf16"),
        ("w1b",   [128, 8, HID],        "bf16"),
        ("w1ab",  [128, 8, HID],        "fp8" if FP8 else "bf16"),
        ("w1pw",  [PW, HID],            "bf16"),
        ("aT",    [128, 8, B_LOC],      "bf16"),
        ("aTr",   [128, 8, 512],        "bf16"),
        ("pwT",   [PW, N_ANTS, B_LOC],  "bf16"),
        ("wout",  [128, 8],             "bf16"),
        ("rough", [B_LOC, N_ANTS],      "f32"),
        ("b1T",   [128, 8],             "f32"),
        ("boutr", [128, 1],             "f32"),
        ("idx",   [128, G // 16],       "i16"),
        ("idxid", [128, B_LOC * 8 // 16], "i16"),
    ]


def _elems(shape, kind):
    n = _prod(shape)
    if kind == "f32":
        return n * 2
    if kind == "fp8":
        assert n % 2 == 0
        return n // 2
    return n


def _blob_offsets():
    offs, off = {}, 0
    for name, shape, kind in _sections():
        offs[name] = off
        off = _align(off + _elems(shape, kind))
    return offs, off


def _build_nc():
    from contextlib import ExitStack

    import concourse.bacc as bacc
    import concourse.mybir as mybir
    import concourse.tile as tile

    f32 = mybir.dt.float32
    bf16 = mybir.dt.bfloat16
    i16 = mybir.dt.int16
    fp8 = mybir.dt.float8e4
    AF = mybir.ActivationFunctionType
    ALU = mybir.AluOpType

    nc = bacc.Bacc("TRN2", target_bir_lowering=False, debug=False,
                   num_devices=CORES)

    offs, blob_len = _blob_offsets()
    blob_d = nc.dram_tensor("blob", [blob_len], bf16, kind="ExternalInput")
    out_d = nc.dram_tensor("out", [B_LOC, 1 + N_ANTS], f32, kind="ExternalOutput")

    _dt = {"bf16": bf16, "f32": f32, "i16": i16, "fp8": fp8}
    _kinds = {name: kind for name, _, kind in _sections()}
    _shapes = {name: shape for name, shape, _ in _sections()}

    def view(name):
        shape, kind = _shapes[name], _kinds[name]
        ap = blob_d[offs[name]:offs[name] + _elems(shape, kind)]
        if kind != "bf16":
            ap = ap.bitcast(_dt[kind])
        if len(shape) == 1:
            return ap
        dims = "abcde"[:len(shape)]
        kw = {d: s for d, s in zip(dims, shape)}
        return ap.rearrange(f"({' '.join(dims)}) -> {' '.join(dims)}", **kw)

    with tile.TileContext(nc) as tc, ExitStack() as ctx:
        const = ctx.enter_context(tc.tile_pool(name="const", bufs=1))
        dram = ctx.enter_context(tc.tile_pool(name="dram", bufs=1, space="DRAM"))
        gpool = ctx.enter_context(tc.tile_pool(name="gather", bufs=2))
        wpool = ctx.enter_context(tc.tile_pool(name="work", bufs=2))
        hpool = ctx.enter_context(tc.tile_pool(name="hsb", bufs=2))
        plp_cm = tc.tile_pool(name="prolog", bufs=1)
        plp = plp_cm.__enter__()
        pp = ctx.enter_context(tc.tile_pool(name="psum", bufs=5, space="PSUM"))
        fp = ctx.enter_context(tc.tile_pool(name="fpsum", bufs=3, space="PSUM"))

        def load(name, dt, eng=None, pool=None):
            shape = _shapes[name]
            t = (pool or const).tile(shape, dt, tag=name)
            (eng or nc.sync).dma_start(t[:], view(name))
            return t

        # load order matters: idx tiles + small consts first so block-0
        # gathers and matmuls can start while the big weights stream in.
        # Big weights go on the scalar HWDGE queue, small/critical on sync.
        idx_sb = load("idx", i16)
        idxid_sb = load("idxid", i16)
        ident = load("ident", bf16)
        w1b = load("w1b", bf16, pool=plp)
        aT = load("aT", bf16, eng=nc.scalar)
        aTr = load("aTr", bf16, eng=nc.scalar)
        wout = load("wout", bf16, eng=nc.scalar)
        b1T = load("b1T", f32, eng=nc.scalar)
        bout = load("boutr", f32, eng=nc.scalar)
        rough = load("rough", f32, eng=nc.scalar)
        w1ab = load("w1ab", fp8 if FP8 else bf16, eng=nc.scalar)
        pwT = load("pwT", bf16, eng=nc.scalar)
        w1pw = load("w1pw", bf16, eng=nc.scalar)
        w1a = load("w1a", bf16, eng=nc.scalar, pool=plp)

        am_ap = view("am")

        # ---- BM = all_mentions @ W1_b for my 1024-row shard, then AllGather.
        # amT: my shard of all_mentions, feature-major, via identity gather.
        # Split into 512-idx halves: one gather's descriptors must fit the
        # SWDGE ring carveout (1024-idx transpose gathers fault on HW).
        amT_h = [plp.tile([128, 8, 256], bf16, tag=f"amT{h}", name=f"amT{h}")
                 for h in range(4)]

        bm_shard = dram.tile([B_LOC * 8, HID], bf16)

        for r in range(DBG_BM_REPEAT):
            for h in range(4):
                nc.gpsimd.dma_gather(amT_h[h][:], am_ap,
                                     idxid_sb[:, h * 16:(h + 1) * 16],
                                     256, 256, EMB, transpose=True)

            for mi in range(8):
                for nh in range(2):
                    ps = pp.tile([128, 512], f32, tag="hp")
                    amT = amT_h[mi // 2]
                    mo = (mi % 2) * 128
                    for ce in range(8):
                        nc.tensor.matmul(
                            ps[:],
                            amT[:, ce, mo:mo + 128],
                            w1b[:, ce, nh * 512:(nh + 1) * 512],
                            start=(ce == 0), stop=(ce == 7))
                    # bf16 convert on the (idle) DVE, small 2-buf bounce tile
                    bmc = plp.tile([128, 512], bf16, tag="bmc", bufs=2)
                    nc.vector.tensor_scalar_mul(bmc[:], ps[:], 1.0)
                    nc.sync.dma_start(
                        bm_shard[mi * 128:(mi + 1) * 128,
                                 nh * 512:(nh + 1) * 512], bmc[:])

        for r in range(DBG_AG_REPEAT):
            bm_full = dram.tile([N_MENT, HID], bf16, addr_space="Shared",
                                name=f"bm_full{r}")
            if not DBG_NO_COLL:
                nc.gpsimd.collective_compute(
                    "AllGather", ALU.bypass,
                    replica_groups=[list(range(CORES))],
                    ins=[bm_shard.opt()],
                    outs=[bm_full.opt()],
                )
            else:
                nc.sync.dma_start(bm_full[0:1024, :], bm_shard[:])

        # ---- A1^T = (a @ W1_a + b1)^T : [h%128, h//128, m], bf16,
        # replicated x4 along cols so the per-block add is one N=512 matmul.
        a1Tr = const.tile([128, 8, 512], bf16)
        for ch in range(8):
            ps = pp.tile([128, B_LOC], f32, tag="hp")
            for ce in range(8):
                nc.tensor.matmul(ps[:], w1a[:, ce, ch * 128:(ch + 1) * 128],
                                 aT[:, ce, :], start=(ce == 0), stop=(ce == 7))
            for j in range(4):
                nc.vector.tensor_scalar_add(a1Tr[:, ch, j * 128:(j + 1) * 128],
                                            ps[:], b1T[:, ch:ch + 1])

        plp_cm.__exit__(None, None, None)
        # staged-block tiles live only after the prologue pool closes -
        # opened here so the allocator can reuse plp's bytes
        spool = ctx.enter_context(tc.tile_pool(name="stage", bufs=1))

        # ---- main loop over pair-column blocks
        scores = const.tile([B_LOC, 1 + N_ANTS], f32)
        nc.vector.memset(scores[:, 0:1], EPS)

        nblk = N_BLK if DBG_NBLK == 0 else max(DBG_NBLK, 0)
        PREF = 4   # bT gathers issued this many blocks ahead of bmT (the
        # bmT gather stalls the FIFO gather queue until the AllGather lands)
        blocks = [b for _ in range(DBG_REPEAT) for b in range(nblk)]

        def block_geom(B):
            nj = min(4, N_ANTS - B * 4)
            return nj, nj * 128, B * (COLS_PER_BLK // 16)

        def issue_bT(B):
            nj, NCOL, i0 = block_geom(B)
            t = gpool.tile([128, 8, NCOL], bf16, tag="bT", name=f"bT{B}", bufs=4)
            if DBG_NO_BT:
                # timing variant: same bytes via plain contiguous HWDGE DMA
                st = B * 128 * 8 * NCOL
                nc.sync.dma_start(t[:], blob_d[st:st + 128 * 8 * NCOL]
                                  .rearrange("(p c n) -> p c n", p=128, c=8))
            else:
                nc.gpsimd.dma_gather(t[:], am_ap,
                                     idx_sb[:, i0:i0 + NCOL // 16],
                                     NCOL, NCOL, EMB, transpose=True)
            return t

        bT_q = {}
        for i, B in enumerate(blocks[:PREF]):
            bT_q[i] = issue_bT(B)

        # bmT gather -> ba = bmT + A1^T and abT = bT * aT are produced one
        # block AHEAD of PE consumption, so the PE's in-order queue never
        # waits on a same-block DVE/gather product.
        ab_q = {}
        no_b = DBG_NO_BMT or DBG_NO_BA
        # staged blocks run ab+pw during the AllGather, b-term combined later
        stage_n = 0 if no_b else min(STAGE, max(0, len(blocks) - 1))

        def issue_ba(bi2, tag="ba", bufs=None):
            B2 = blocks[bi2]
            nj2, NCOL2, i02 = block_geom(B2)
            bmT = gpool.tile([128, 8, NCOL2], bf16, tag="bmT")
            nc.gpsimd.dma_gather(bmT[:], bm_full[:],
                                 idx_sb[:, i02:i02 + NCOL2 // 16],
                                 NCOL2, NCOL2, HID, transpose=True)
            ba = wpool.tile([128, 8, NCOL2], bf16, tag=tag, bufs=bufs)
            nc.vector.tensor_add(ba[:], bmT[:], a1Tr[:, :, 0:NCOL2])
            return ba

        def issue_ab(bi2):
            B2 = blocks[bi2]
            nj2, NCOL2, i02 = block_geom(B2)
            abT = wpool.tile([128, 8, NCOL2], fp8 if FP8 else bf16, tag="abT")
            nc.vector.tensor_mul(abT[:], bT_q[bi2][:], aTr[:, :, 0:NCOL2])
            ba = None
            if not no_b and bi2 >= stage_n:
                ba = issue_ba(bi2)
            ab_q[bi2] = (abT, ba)

        issue_ab(0)

        # staged pipeline state: s_wait -> (gather+ba) -> s_ready ->
        # (DVE add + scalar lrelu) -> s_comb -> (PE layer-2 + scores)
        s_wait = list(range(stage_n))
        s_ready = []   # (bi, ba_tile)
        s_comb = []    # (bi, hT_tile)
        hpre = {}      # bi -> staged ab+pw SBUF tile

        # Per block: PE runs [4x ab DoubleRow + pw + identity-add(ba)] per
        # h-chunk; the b-term lands via the identity matmul (rhs = ba =
        # bmT + A1^T, precomputed on DVE with no dependence on this block's
        # psum), and layer-2 matmuls are DEFERRED one block so the in-order
        # PE queue never waits on this block's lrelu output.
        prev = None  # (hT, ffps, nj, j0) of previous block

        def issue_l2(pv, ch):
            hT_p, ffps_p, nj_p, _ = pv
            for j in range(nj_p):
                nc.tensor.matmul(ffps_p[:, j:j + 1],
                                 hT_p[:, ch, j * 128:(j + 1) * 128],
                                 wout[:, ch:ch + 1],
                                 start=(ch == 0 and j == 0),
                                 stop=(ch == 7 and j == nj_p - 1))

        def finish_scores(pv):
            _, ffps_p, nj_p, j0_p = pv
            # scores[:, 1+j0 : 1+j0+nj] = ffnn + b_out + rough
            nc.vector.scalar_tensor_tensor(scores[:, 1 + j0_p:1 + j0_p + nj_p],
                                           ffps_p[:], bout[:, 0:1],
                                           rough[:, j0_p:j0_p + nj_p],
                                           op0=ALU.add, op1=ALU.add)

        def staged_step():
            # one pipeline advance; each staged block gets >= one normal
            # iteration between its stages, so PE/DVE/scalar never wait on
            # a same-iteration product.
            if s_comb:
                s0, hT_s = s_comb.pop(0)
                nj0, NCOL0, _ = block_geom(blocks[s0])
                ffps_s = fp.tile([128, nj0], f32, tag="ff")
                for ch in range(8):
                    for j in range(nj0):
                        nc.tensor.matmul(ffps_s[:, j:j + 1],
                                         hT_s[:, ch, j * 128:(j + 1) * 128],
                                         wout[:, ch:ch + 1],
                                         start=(ch == 0 and j == 0),
                                         stop=(ch == 7 and j == nj0 - 1))
                finish_scores((None, ffps_s, nj0, blocks[s0] * 4))
            if s_ready:
                s1, ba_s = s_ready.pop(0)
                nj1, NCOL1, _ = block_geom(blocks[s1])
                t = spool.tile([128, 8, NCOL1], bf16, tag="ts", bufs=1)
                nc.vector.tensor_add(t[:], hpre.pop(s1)[:], ba_s[:])
                hT_s = spool.tile([128, 8, NCOL1], bf16, tag="hTs", bufs=2)
                nc.scalar.activation(hT_s[:], t[:], AF.Lrelu, alpha=0.01)
                s_comb.append((s1, hT_s))
            if s_wait:
                s2 = s_wait.pop(0)
                s_ready.append((s2, issue_ba(s2, tag="bas", bufs=1)))

        for bi, B in enumerate(blocks):
            nj, NCOL, i0 = block_geom(B)
            j0 = B * 4
            staged = bi < stage_n

            if bi + PREF < len(blocks):
                bT_q[bi + PREF] = issue_bT(blocks[bi + PREF])
            abT, ba = ab_q.pop(bi)
            bT_q.pop(bi)
            if bi + 1 < len(blocks):
                issue_ab(bi + 1)
            use_ba = ba is not None

            if not staged:
                ffps = fp.tile([128, nj], f32, tag="ff")
                hT = hpool.tile([128, 8, NCOL], bf16, tag="hT")
            else:
                hpre[bi] = spool.tile([128, 8, NCOL], bf16, tag=f"hpre{bi}",
                                      name=f"hpre{bi}", bufs=1)
            for ch in range(8):
                hp = pp.tile([128, NCOL], f32, tag="hp")
                if FP8:
                    # DoubleRow: 2 K-chunks of 128 per instruction, 2 rows
                    # per PE cell - halves both stream cycles and instr count
                    for cp in range(4):
                        nc.tensor.matmul(
                            hp[:],
                            w1ab[:, 2 * cp:2 * cp + 2, ch * 128:(ch + 1) * 128],
                            abT[:, 2 * cp:2 * cp + 2, :],
                            start=(cp == 0), stop=False,
                            perf_mode=mybir.MatmulPerfMode.DoubleRow)
                else:
                    for ce in range(8):
                        nc.tensor.matmul(hp[:],
                                         w1ab[:, ce, ch * 128:(ch + 1) * 128],
                                         abT[:, ce, :], start=(ce == 0), stop=False)
                nc.tensor.matmul(hp[:], w1pw[:, ch * 128:(ch + 1) * 128],
                                 pwT[:, j0:j0 + nj, :], start=False,
                                 stop=not use_ba)
                if use_ba:
                    nc.tensor.matmul(hp[:], ident[:],
                                     ba[:, ch, :], start=False, stop=True)
                if not staged and prev is not None:
                    issue_l2(prev, ch)

                if staged:
                    # park ab+pw in SBUF; b-term combined post-AllGather
                    nc.scalar.activation(hpre[bi][:, ch, :], hp[:], AF.Copy)
                elif SIM_SAFE or DBG_NO_LRELU:  # CoreSim lacks Lrelu
                    lt = hpool.tile([128, NCOL], f32, tag="lt")
                    nc.vector.tensor_scalar_mul(lt[:], hp[:], 0.01)
                    nc.vector.tensor_max(hT[:, ch, :], hp[:], lt[:])
                else:
                    nc.scalar.activation(hT[:, ch, :], hp[:], AF.Lrelu, alpha=0.01)

            if not staged:
                if prev is not None:
                    finish_scores(prev)
                prev = (hT, ffps, nj, j0)
                staged_step()

        # epilogue: layer-2 + scores for the final normal block, then drain
        # the staged pipeline
        if prev is not None:
            for ch in range(8):
                issue_l2(prev, ch)
            finish_scores(prev)
        while s_wait or s_ready or s_comb:
            staged_step()

        nc.sync.dma_start(out_d[:], scores[:])

    nc.compile()
    return nc


def _shard(inputs):
    am = np.asarray(inputs["all_mentions"], np.float32)
    mb_ = np.asarray(inputs["mentions_batch"], np.float32)
    pw = np.asarray(inputs["pw_batch"], np.float32)
    rough = np.asarray(inputs["top_rough_scores_batch"], np.float32)
    W1 = np.asarray(inputs["W1"], np.float32)
    b1 = np.asarray(inputs["b1"], np.float32)
    Wout = np.asarray(inputs["W_out"], np.float32)
    bout = np.asarray(inputs["b_out"], np.float32)
    idx = np.asarray(inputs["top_indices_batch"])

    am_bf = am.astype(_BF16)
    S = FP8_SCALE if FP8 else 1.0

    def wblock(rows, dt=_BF16):  # [1024, 1024] -> [128, 8, 1024] (e%128, e//128, h)
        return np.ascontiguousarray(
            rows.reshape(8, 128, HID).transpose(1, 0, 2)).astype(dt)

    w1a = wblock(S * W1[0:1024])
    w1b = wblock(S * W1[1024:2048])
    w1ab = wblock(S * W1[2048:3072],
                  ml_dtypes.float8_e4m3fn if FP8 else _BF16)
    w1pw = np.ascontiguousarray(S * W1[3072:3136]).astype(_BF16)      # [64, 1024]
    wout = np.ascontiguousarray(
        Wout[:, 0].reshape(8, 128).T / S).astype(_BF16)
    b1T = np.ascontiguousarray(S * b1.reshape(8, 128).T).astype(np.float32)
    boutr = np.full((128, 1), bout[0], np.float32)

    def wrap_idx(flat, pad_cols):
        # [16, n/16] wrapped block, replicated across all 8 GPSIMD-core
        # partition groups (the odd Q7 core reads partitions 16-31).
        return np.tile(flat.reshape(pad_cols, 16).T, (8, 1))

    def as_u16(a):
        a = np.ascontiguousarray(a)
        if a.dtype.itemsize == 1:
            a = a.reshape(-1)
            assert a.size % 2 == 0
            return a.view(np.uint16)
        return a.view(np.uint16).reshape(-1)

    in_maps = []
    for c in range(CORES):
        sl = slice(c * B_LOC, (c + 1) * B_LOC)
        aT = np.ascontiguousarray(
            mb_[sl].T.reshape(8, 128, B_LOC).transpose(1, 0, 2)).astype(_BF16)
        aTr = np.ascontiguousarray(np.tile(aT, (1, 1, 4)))
        pwT = np.ascontiguousarray(pw[sl].transpose(2, 1, 0)).astype(_BF16)
        idx_flat = np.ascontiguousarray(idx[sl].astype(np.int16).T).reshape(G)
        idxid_flat = np.arange(c * B_LOC * 8, (c + 1) * B_LOC * 8, dtype=np.int16)
        sections = {
            "am": am_bf,
            "ident": np.eye(128, dtype=_BF16),
            "aT": aT,
            "aTr": aTr,
            "pwT": pwT,
            "rough": np.ascontiguousarray(rough[sl]),
            "w1a": w1a, "w1b": w1b, "w1ab": w1ab, "w1pw": w1pw,
            "wout": wout, "b1T": b1T, "boutr": boutr,
            "idx": wrap_idx(idx_flat, G // 16),
            "idxid": wrap_idx(idxid_flat, B_LOC * 8 // 16),
        }
        offs, blob_len = _blob_offsets()
        blob = np.zeros(blob_len, np.uint16)
        for name, shape, kind in _sections():
            u = as_u16(sections[name])
            blob[offs[name]:offs[name] + u.size] = u
        in_maps.append({"blob": blob.view(_BF16)})
    return in_maps


def get_nc():
    if "nc" not in _cache:
        _cache["nc"] = _build_nc()
    return _cache["nc"]


def kernel(**inputs):
    import time

    from concourse.bass_utils import run_bass_kernel_spmd

    nc = get_nc()
    in_maps = _shard(inputs)
    try:
        res = run_bass_kernel_spmd(nc, in_maps, core_ids=list(range(CORES)))
    except Exception:  # transient axon/NRT wedge: one retry
        time.sleep(20)
        res = run_bass_kernel_spmd(nc, in_maps, core_ids=list(range(CORES)))
    outs = [r["out"] for r in res.results]
    return np.concatenate(outs, axis=0)


# revision 46
# speedup vs baseline: 1.2858x; 1.0161x over previous
"""Trainium2 Bass kernel for the AnaphoricityScorer (coref pair scorer).

Reference computation (per mention row m, antecedent slot j):
    b = all_mentions[idx[m, j]]                       # gather, 1024-dim
    pair = [a_m, b, a_m * b, pw[m, j]]                # 3136-dim
    h = leaky_relu(pair @ W1 + b1)                    # 1024-dim
    score[m, j] = rough[m, j] + h @ W_out + b_out
    out[m] = [EPS, score[m, 0..49]]                   # [1024, 51]

Strategy (8 NeuronCores, data-parallel over the 1024-mention batch):
  * 128 mentions per core; all_mentions + FFNN weights replicated.
  * Decompose W1 by pair-feature block: W1 = [W1_a; W1_b; W1_ab; W1_pw].
      - a-term:  A1^T = (W1_a^T a^T) computed once per core (tiny).
      - b-term:  BM = all_mentions @ W1_b precomputed SHARDED across the 8
        cores (1024 rows each) + AllGather, then per-pair rows of BM are
        *gathered* instead of re-multiplied (saves 13.4 GFLOP/core).
      - ab-term: full matmul (a (x) b)^T @ ... -> irreducible 13.4 GFLOP/core,
        run in fp8e4m3 with MatmulPerfMode.DoubleRow (2 K-chunks of 128 per
        instruction, ~2x stream rate; measured 2419 cyc vs bf16 4096 per
        K=1024 x N=512 group). W1 blocks carry a x64 scale (keeps e4m3 out of
        subnormals); leaky-relu is positively homogeneous, so /64 baked into
        W_out cancels it. rel err 1.0e-2 vs 1.8e-3 in bf16 (gate 2e-2).
      - pw-term: K=64 bf16 matmul.
  * dma_gather(transpose=True) both gathers antecedent rows AND delivers them
    feature-major ([emb%128, emb//128, paircol]) - exactly the moving-operand
    layout the matmul needs. Pair columns are ordered g = j*128 + m so each
    128-column group shares one antecedent slot j across all 128 mentions.
  * Per 512-column block and h-chunk, the PE stream is [4x ab DoubleRow + pw
    + identity-matmul(ba)]: the b-term enters PSUM via a PE identity matmul
    whose rhs ba = bmT + A1^T is precomputed on the DVE one block AHEAD
    (bmT gather + DVE add prefetched), and layer-2 (N=1 matmuls vs W_out)
    is DEFERRED one block - so the in-order PE queue never waits on a
    same-block DVE/scalar/gather product (DVE adds into PSUM measured
    3.1us/block exposed in the fused form). leaky-relu on the ScalarEngine
    (Lrelu) -> bf16 h^T consumed by next block's layer-2.
  * bT gathers prefetched 4 blocks ahead (the first bmT gather stalls the
    FIFO gather queue until the AllGather lands).
  * ALL inputs are packed into ONE bf16 DRAM blob per core (f32/i16/fp8
    sections bitcast) - the axon per-execute dispatch cost scales with the
    input tensor count (~60-120us per input x device), so 17 inputs -> 1
    (measured -870us/exec head-to-head).
"""

import numpy as np
import ml_dtypes

CORES = 8
BATCH = 1024
B_LOC = BATCH // CORES          # 128 mentions per core
N_ANTS = 50
EMB = 1024
PW = 64
HID = 1024
N_MENT = 8192
G = B_LOC * N_ANTS              # 6400 pair columns per core
EPS = 1e-7

COLS_PER_BLK = 512              # pair columns per block (4 antecedent slots)
N_BLK = (G + COLS_PER_BLK - 1) // COLS_PER_BLK   # 13 (12 full + 1 half)

SIM_SAFE = False                # set True before get_nc() for CoreSim runs
import os as _os
DBG_NO_COLL = _os.environ.get("KDBG_NO_COLL", "0") == "1"   # skip AllGather (wrong results)
DBG_NBLK = int(_os.environ.get("KDBG_NBLK", "0"))           # limit main-loop blocks
DBG_NO_LRELU = _os.environ.get("KDBG_NO_LRELU", "0") == "1"
DBG_REPEAT = int(_os.environ.get("KDBG_REPEAT", "1"))  # repeat main loop (timing)
DBG_AG_REPEAT = int(_os.environ.get("KDBG_AG_REPEAT", "1"))  # repeat AllGather
DBG_BM_REPEAT = int(_os.environ.get("KDBG_BM_REPEAT", "1"))  # repeat BM compute
DBG_NO_BT = _os.environ.get("KDBG_NO_BT", "0") == "1"    # skip bT gather (wrong)
DBG_NO_BMT = _os.environ.get("KDBG_NO_BMT", "0") == "1"  # skip bmT gather+add (wrong)
DBG_NO_BA = _os.environ.get("KDBG_NO_BA", "0") == "1"    # keep gather, skip adds (wrong)
FP8 = _os.environ.get("KDBG_FP8", "1") == "1"  # ab-term matmuls in fp8e4m3
# Blocks whose ab+pw psum is staged to SBUF during the AllGather; their
# b-term is combined later on DVE/scalar while the normal loop runs on PE.
STAGE = int(_os.environ.get("KDBG_STAGE", "3"))
FP8_SCALE = 64.0  # keeps 64*W1ab out of the e4m3 subnormal range; lrelu is
# positively homogeneous so baking x64 into W1 and /64 into W_out cancels

_BF16 = ml_dtypes.bfloat16

_cache = {}


def _prod(s):
    p = 1
    for x in s:
        p *= x
    return p


def _align(x, a=256):
    return (x + a - 1) // a * a


# ---- blob layout: (name, shape, elem-kind). Kinds: bf16/i16 (1 blob elem),
# f32 (2 blob elems), fp8 (1/2 blob elem). Offsets in bf16 elems, 256-aligned.
def _sections():
    return [
        ("am",    [N_MENT, EMB],        "bf16"),
        ("ident", [128, 128],           "bf16"),
        ("w1a",   [128, 8, HID],        "bf16"),
        ("w1b",   [128, 8, HID],        "bf16"),
        ("w1ab",  [128, 8, HID],        "fp8" if FP8 else "bf16"),
        ("w1pw",  [PW, HID],            "bf16"),
        ("aT",    [128, 8, B_LOC],      "bf16"),
        ("aTr",   [128, 8, 512],        "bf16"),
        ("pwT",   [PW, N_ANTS, B_LOC],  "bf16"),
        ("wout",  [128, 8],             "bf16"),
        ("rough", [B_LOC, N_ANTS],      "f32"),
        ("b1T",   [128, 8],             "f32"),
        ("boutr", [128, 1],             "f32"),
        ("idx",   [128, G // 16],       "i16"),
        ("idxid", [128, B_LOC * 8 // 16], "i16"),
    ]


def _elems(shape, kind):
    n = _prod(shape)
    if kind == "f32":
        return n * 2
    if kind == "fp8":
        assert n % 2 == 0
        return n // 2
    return n


def _blob_offsets():
    offs, off = {}, 0
    for name, shape, kind in _sections():
        offs[name] = off
        off = _align(off + _elems(shape, kind))
    return offs, off


def _build_nc():
    from contextlib import ExitStack

    import concourse.bacc as bacc
    import concourse.mybir as mybir
    import concourse.tile as tile

    f32 = mybir.dt.float32
    bf16 = mybir.dt.bfloat16
    i16 = mybir.dt.int16
    fp8 = mybir.dt.float8e4
    AF = mybir.ActivationFunctionType
    ALU = mybir.AluOpType

    nc = bacc.Bacc("TRN2", target_bir_lowering=False, debug=False,
                   num_devices=CORES)

    offs, blob_len = _blob_offsets()
    blob_d = nc.dram_tensor("blob", [blob_len], bf16, kind="ExternalInput")
    out_d = nc.dram_tensor("out", [B_LOC, 1 + N_ANTS], f32, kind="ExternalOutput")

    _dt = {"bf16": bf16, "f32": f32, "i16": i16, "fp8": fp8}
    _kinds = {name: kind for name, _, kind in _sections()}
    _shapes = {name: shape for name, shape, _ in _sections()}

    def view(name):
        shape, kind = _shapes[name], _kinds[name]
        ap = blob_d[offs[name]:offs[name] + _elems(shape, kind)]
        if kind != "bf16":
            ap = ap.bitcast(_dt[kind])
        if len(shape) == 1:
            return ap
        dims = "abcde"[:len(shape)]
        kw = {d: s for d, s in zip(dims, shape)}
        return ap.rearrange(f"({' '.join(dims)}) -> {' '.join(dims)}", **kw)

    with tile.TileContext(nc) as tc, ExitStack() as ctx:
        const = ctx.enter_context(tc.tile_pool(name="const", bufs=1))
        dram = ctx.enter_context(tc.tile_pool(name="dram", bufs=1, space="DRAM"))
        gpool = ctx.enter_context(tc.tile_pool(name="gather", bufs=2))
        wpool = ctx.enter_context(tc.tile_pool(name="work", bufs=2))
        hpool = ctx.enter_context(tc.tile_pool(name="hsb", bufs=2))
        plp_cm = tc.tile_pool(name="prolog", bufs=1)
        plp = plp_cm.__enter__()
        pp = ctx.enter_context(tc.tile_pool(name="psum", bufs=5, space="PSUM"))
        fp = ctx.enter_context(tc.tile_pool(name="fpsum", bufs=3, space="PSUM"))

        def load(name, dt, eng=None, pool=None):
            shape = _shapes[name]
            t = (pool or const).tile(shape, dt, tag=name)
            (eng or nc.sync).dma_start(t[:], view(name))
            return t

        # load order matters: idx tiles + small consts first so block-0
        # gathers and matmuls can start while the big weights stream in.
        # Big weights go on the scalar HWDGE queue, small/critical on sync.
        idx_sb = load("idx", i16)
        idxid_sb = load("idxid", i16)
        ident = load("ident", bf16)
        w1b = load("w1b", bf16, pool=plp)
        aT = load("aT", bf16, eng=nc.scalar)
        aTr = load("aTr", bf16, eng=nc.scalar)
        wout = load("wout", bf16, eng=nc.scalar)
        b1T = load("b1T", f32, eng=nc.scalar)
        bout = load("boutr", f32, eng=nc.scalar)
        rough = load("rough", f32, eng=nc.scalar)
        w1ab = load("w1ab", fp8 if FP8 else bf16, eng=nc.scalar)
        pwT = load("pwT", bf16, eng=nc.scalar)
        w1pw = load("w1pw", bf16, eng=nc.scalar)
        w1a = load("w1a", bf16, eng=nc.scalar, pool=plp)

        am_ap = view("am")

        # ---- BM = all_mentions @ W1_b for my 1024-row shard, then AllGather.
        # amT: my shard of all_mentions, feature-major, via identity gather.
        # Split into 512-idx halves: one gather's descriptors must fit the
        # SWDGE ring carveout (1024-idx transpose gathers fault on HW).
        amT_h = [plp.tile([128, 8, 256], bf16, tag=f"amT{h}", name=f"amT{h}")
                 for h in range(4)]

        bm_shard = dram.tile([B_LOC * 8, HID], bf16)

        for r in range(DBG_BM_REPEAT):
            for h in range(4):
                nc.gpsimd.dma_gather(amT_h[h][:], am_ap,
                                     idxid_sb[:, h * 16:(h + 1) * 16],
                                     256, 256, EMB, transpose=True)

            for mi in range(8):
                for nh in range(2):
                    ps = pp.tile([128, 512], f32, tag="hp")
                    amT = amT_h[mi // 2]
                    mo = (mi % 2) * 128
                    for ce in range(8):
                        nc.tensor.matmul(
                            ps[:],
                            amT[:, ce, mo:mo + 128],
                            w1b[:, ce, nh * 512:(nh + 1) * 512],
                            start=(ce == 0), stop=(ce == 7))
                    # bf16 convert on the (idle) DVE, small 2-buf bounce tile
                    bmc = plp.tile([128, 512], bf16, tag="bmc", bufs=2)
                    nc.vector.tensor_scalar_mul(bmc[:], ps[:], 1.0)
                    nc.sync.dma_start(
                        bm_shard[mi * 128:(mi + 1) * 128,
                                 nh * 512:(nh + 1) * 512], bmc[:])

        for r in range(DBG_AG_REPEAT):
            bm_full = dram.tile([N_MENT, HID], bf16, addr_space="Shared",
                                name=f"bm_full{r}")
            if not DBG_NO_COLL:
                nc.gpsimd.collective_compute(
                    "AllGather", ALU.bypass,
                    replica_groups=[list(range(CORES))],
                    ins=[bm_shard.opt()],
                    outs=[bm_full.opt()],
                )
            else:
                nc.sync.dma_start(bm_full[0:1024, :], bm_shard[:])

        # ---- A1^T = (a @ W1_a + b1)^T : [h%128, h//128, m], bf16,
        # replicated x4 along cols so the per-block add is one N=512 matmul.
        a1Tr = const.tile([128, 8, 512], bf16)
        for ch in range(8):
            ps = pp.tile([128, B_LOC], f32, tag="hp")
            for ce in range(8):
                nc.tensor.matmul(ps[:], w1a[:, ce, ch * 128:(ch + 1) * 128],
                                 aT[:, ce, :], start=(ce == 0), stop=(ce == 7))
            for j in range(4):
                nc.vector.tensor_scalar_add(a1Tr[:, ch, j * 128:(j + 1) * 128],
                                            ps[:], b1T[:, ch:ch + 1])

        plp_cm.__exit__(None, None, None)
        # staged-block tiles live only after the prologue pool closes -
        # opened here so the allocator can reuse plp's bytes
        spool = ctx.enter_context(tc.tile_pool(name="stage", bufs=1))

        # ---- main loop over pair-column blocks
        scores = const.tile([B_LOC, 1 + N_ANTS], f32)
        nc.vector.memset(scores[:, 0:1], EPS)

        nblk = N_BLK if DBG_NBLK == 0 else max(DBG_NBLK, 0)
        PREF = 4   # bT gathers issued this many blocks ahead of bmT (the
        # bmT gather stalls the FIFO gather queue until the AllGather lands)
        blocks = [b for _ in range(DBG_REPEAT) for b in range(nblk)]

        def block_geom(B):
            nj = min(4, N_ANTS - B * 4)
            return nj, nj * 128, B * (COLS_PER_BLK // 16)

        def issue_bT(B):
            nj, NCOL, i0 = block_geom(B)
            t = gpool.tile([128, 8, NCOL], bf16, tag="bT", name=f"bT{B}", bufs=4)
            if DBG_NO_BT:
                # timing variant: same bytes via plain contiguous HWDGE DMA
                st = B * 128 * 8 * NCOL
                nc.sync.dma_start(t[:], blob_d[st:st + 128 * 8 * NCOL]
                                  .rearrange("(p c n) -> p c n", p=128, c=8))
            else:
                nc.gpsimd.dma_gather(t[:], am_ap,
                                     idx_sb[:, i0:i0 + NCOL // 16],
                                     NCOL, NCOL, EMB, transpose=True)
            return t

        bT_q = {}
        for i, B in enumerate(blocks[:PREF]):
            bT_q[i] = issue_bT(B)

        # bmT gather -> ba = bmT + A1^T and abT = bT * aT are produced one
        # block AHEAD of PE consumption, so the PE's in-order queue never
        # waits on a same-block DVE/gather product.
        ab_q = {}
        no_b = DBG_NO_BMT or DBG_NO_BA
        # staged blocks run ab+pw during the AllGather, b-term combined later
        stage_n = 0 if no_b else min(STAGE, max(0, len(blocks) - 1))

        def issue_ba(bi2, tag="ba", bufs=None):
            B2 = blocks[bi2]
            nj2, NCOL2, i02 = block_geom(B2)
            bmT = gpool.tile([128, 8, NCOL2], bf16, tag="bmT")
            nc.gpsimd.dma_gather(bmT[:], bm_full[:],
                                 idx_sb[:, i02:i02 + NCOL2 // 16],
                                 NCOL2, NCOL2, HID, transpose=True)
            ba = wpool.tile([128, 8, NCOL2], bf16, tag=tag, bufs=bufs)
            nc.vector.tensor_add(ba[:], bmT[:], a1Tr[:, :, 0:NCOL2])
            return ba

        def issue_ab(bi2):
            B2 = blocks[bi2]
            nj2, NCOL2, i02 = block_geom(B2)
            abT = wpool.tile([128, 8, NCOL2], fp8 if FP8 else bf16, tag="abT")
            nc.vector.tensor_mul(abT[:], bT_q[bi2][:], aTr[:, :, 0:NCOL2])
            ba = None
            if not no_b and bi2 >= stage_n:
                ba = issue_ba(bi2)
            ab_q[bi2] = (abT, ba)

        issue_ab(0)

        # staged pipeline state: s_wait -> (gather+ba) -> s_ready ->
        # (DVE add + scalar lrelu) -> s_comb -> (PE layer-2 + scores)
        s_wait = list(range(stage_n))
        s_ready = []   # (bi, ba_tile)
        s_comb = []    # (bi, hT_tile)
        hpre = {}      # bi -> staged ab+pw SBUF tile

        # Per block: PE runs [4x ab DoubleRow + pw + identity-add(ba)] per
        # h-chunk; the b-term lands via the identity matmul (rhs = ba =
        # bmT + A1^T, precomputed on DVE with no dependence on this block's
        # psum), and layer-2 matmuls are DEFERRED one block so the in-order
        # PE queue never waits on this block's lrelu output.
        prev = None  # (hT, ffps, nj, j0) of previous block

        def issue_l2(pv, ch):
            hT_p, ffps_p, nj_p, _ = pv
            for j in range(nj_p):
                nc.tensor.matmul(ffps_p[:, j:j + 1],
                                 hT_p[:, ch, j * 128:(j + 1) * 128],
                                 wout[:, ch:ch + 1],
                                 start=(ch == 0 and j == 0),
                                 stop=(ch == 7 and j == nj_p - 1))

        def finish_scores(pv):
            _, ffps_p, nj_p, j0_p = pv
            # scores[:, 1+j0 : 1+j0+nj] = ffnn + b_out + rough
            nc.vector.scalar_tensor_tensor(scores[:, 1 + j0_p:1 + j0_p + nj_p],
                                           ffps_p[:], bout[:, 0:1],
                                           rough[:, j0_p:j0_p + nj_p],
                                           op0=ALU.add, op1=ALU.add)

        def staged_step():
            # one pipeline advance; each staged block gets >= one normal
            # iteration between its stages, so PE/DVE/scalar never wait on
            # a same-iteration product.
            if s_comb:
                s0, hT_s = s_comb.pop(0)
                nj0, NCOL0, _ = block_geom(blocks[s0])
                ffps_s = fp.tile([128, nj0], f32, tag="ff")
                for ch in range(8):
                    for j in range(nj0):
                        nc.tensor.matmul(ffps_s[:, j:j + 1],
                                         hT_s[:, ch, j * 128:(j + 1) * 128],
                                         wout[:, ch:ch + 1],
                                         start=(ch == 0 and j == 0),
                                         stop=(ch == 7 and j == nj0 - 1))
                finish_scores((None, ffps_s, nj0, blocks[s0] * 4))
            if s_ready:
                s1, ba_s = s_ready.pop(0)
                nj1, NCOL1, _ = block_geom(blocks[s1])
                t = spool.tile([128, 8, NCOL1], bf16, tag="ts", bufs=1)
                nc.vector.tensor_add(t[:], hpre.pop(s1)[:], ba_s[:])
                hT_s = spool.tile([128, 8, NCOL1], bf16, tag="hTs", bufs=2)
                nc.scalar.activation(hT_s[:], t[:], AF.Lrelu, alpha=0.01)
                s_comb.append((s1, hT_s))
            if s_wait:
                s2 = s_wait.pop(0)
                s_ready.append((s2, issue_ba(s2, tag="bas", bufs=1)))

        for bi, B in enumerate(blocks):
            nj, NCOL, i0 = block_geom(B)
            j0 = B * 4
            staged = bi < stage_n

            if bi + PREF < len(blocks):
                bT_q[bi + PREF] = issue_bT(blocks[bi + PREF])
            abT, ba = ab_q.pop(bi)
            bT_q.pop(bi)
            if bi + 1 < len(blocks):
                issue_ab(bi + 1)
            use_ba = ba is not None

            if not staged:
                ffps = fp.tile([128, nj], f32, tag="ff")
                hT = hpool.tile([128, 8, NCOL], bf16, tag="hT")
            else:
                hpre[bi] = spool.tile([128, 8, NCOL], bf16, tag=f"hpre{bi}",
                                      name=f"hpre{bi}", bufs=1)
            for ch in range(8):
                hp = pp.tile([128, NCOL], f32, tag="hp")
                if FP8:
                    # DoubleRow: 2 K-chunks of 128 per instruction, 2 rows
                    # per PE cell - halves both stream cycles and instr count
                    for cp in range(4):
                        nc.tensor.matmul(
                            hp[:],
                            w1ab[:, 2 * cp:2 * cp + 2, ch * 128:(ch + 1) * 128],
                            abT[:, 2 * cp:2 * cp + 2, :],
                            start=(cp == 0), stop=False,
                            perf_mode=mybir.MatmulPerfMode.DoubleRow)
                else:
                    for ce in range(8):
                        nc.tensor.matmul(hp[:],
                                         w1ab[:, ce, ch * 128:(ch + 1) * 128],
                                         abT[:, ce, :], start=(ce == 0), stop=False)
                nc.tensor.matmul(hp[:], w1pw[:, ch * 128:(ch + 1) * 128],
                                 pwT[:, j0:j0 + nj, :], start=False,
                                 stop=not use_ba)
                if use_ba:
                    nc.tensor.matmul(hp[:], ident[:],
                                     ba[:, ch, :], start=False, stop=True)
                if not staged and prev is not None:
                    issue_l2(prev, ch)

                if staged:
                    # park ab+pw in SBUF; b-term combined post-AllGather
                    nc.scalar.activation(hpre[bi][:, ch, :], hp[:], AF.Copy)
                elif SIM_SAFE or DBG_NO_LRELU:  # CoreSim lacks Lrelu
                    lt = hpool.tile([128, NCOL], f32, tag="lt")
                    nc.vector.tensor_scalar_mul(lt[:], hp[:], 0.01)
                    nc.vector.tensor_max(hT[:, ch, :], hp[:], lt[:])
                else:
                    nc.scalar.activation(hT[:, ch, :], hp[:], AF.Lrelu, alpha=0.01)

            if not staged:
                if prev is not None:
                    finish_scores(prev)
                prev = (hT, ffps, nj, j0)
                staged_step()

        # epilogue: layer-2 + scores for the final normal block, then drain
        # the staged pipeline
        if prev is not None:
            for ch in range(8):
                issue_l2(prev, ch)
            finish_scores(prev)
        while s_wait or s_ready or s_comb:
            staged_step()

        nc.sync.dma_start(out_d[:], scores[:])

    nc.compile()
    return nc


def _shard(inputs):
    am = np.asarray(inputs["all_mentions"], np.float32)
    mb_ = np.asarray(inputs["mentions_batch"], np.float32)
    pw = np.asarray(inputs["pw_batch"], np.float32)
    rough = np.asarray(inputs["top_rough_scores_batch"], np.float32)
    W1 = np.asarray(inputs["W1"], np.float32)
    b1 = np.asarray(inputs["b1"], np.float32)
    Wout = np.asarray(inputs["W_out"], np.float32)
    bout = np.asarray(inputs["b_out"], np.float32)
    idx = np.asarray(inputs["top_indices_batch"])

    am_bf = am.astype(_BF16)
    S = FP8_SCALE if FP8 else 1.0

    def wblock(rows, dt=_BF16):  # [1024, 1024] -> [128, 8, 1024] (e%128, e//128, h)
        return np.ascontiguousarray(
            rows.reshape(8, 128, HID).transpose(1, 0, 2)).astype(dt)

    w1a = wblock(S * W1[0:1024])
    w1b = wblock(S * W1[1024:2048])
    w1ab = wblock(S * W1[2048:3072],
                  ml_dtypes.float8_e4m3fn if FP8 else _BF16)
    w1pw = np.ascontiguousarray(S * W1[3072:3136]).astype(_BF16)      # [64, 1024]
    wout = np.ascontiguousarray(
        Wout[:, 0].reshape(8, 128).T / S).astype(_BF16)
    b1T = np.ascontiguousarray(S * b1.reshape(8, 128).T).astype(np.float32)
    boutr = np.full((128, 1), bout[0], np.float32)

    def wrap_idx(flat, pad_cols):
        # [16, n/16] wrapped block, replicated across all 8 GPSIMD-core
        # partition groups (the odd Q7 core reads partitions 16-31).
        return np.tile(flat.reshape(pad_cols, 16).T, (8, 1))

    def as_u16(a):
        a = np.ascontiguousarray(a)
        if a.dtype.itemsize == 1:
            a = a.reshape(-1)
            assert a.size % 2 == 0
            return a.view(np.uint16)
        return a.view(np.uint16).reshape(-1)

    in_maps = []
    for c in range(CORES):
        sl = slice(c * B_LOC, (c + 1) * B_LOC)
        aT = np.ascontiguousarray(
            mb_[sl].T.reshape(8, 128, B_LOC).transpose(1, 0, 2)).astype(_BF16)
        aTr = np.ascontiguousarray(np.tile(aT, (1, 1, 4)))
        pwT = np.ascontiguousarray(pw[sl].transpose(2, 1, 0)).astype(_BF16)
        idx_flat = np.ascontiguousarray(idx[sl].astype(np.int16).T).reshape(G)
        idxid_flat = np.arange(c * B_LOC * 8, (c + 1) * B_LOC * 8, dtype=np.int16)
        sections = {
            "am": am_bf,
            "ident": np.eye(128, dtype=_BF16),
            "aT": aT,
            "aTr": aTr,
            "pwT": pwT,
            "rough": np.ascontiguousarray(rough[sl]),
            "w1a": w1a, "w1b": w1b, "w1ab": w1ab, "w1pw": w1pw,
            "wout": wout, "b1T": b1T, "boutr": boutr,
            "idx": wrap_idx(idx_flat, G // 16),
            "idxid": wrap_idx(idxid_flat, B_LOC * 8 // 16),
        }
        offs, blob_len = _blob_offsets()
        blob = np.zeros(blob_len, np.uint16)
        for name, shape, kind in _sections():
            u = as_u16(sections[name])
            blob[offs[name]:offs[name] + u.size] = u
        in_maps.append({"blob": blob.view(_BF16)})
    return in_maps


def get_nc():
    if "nc" not in _cache:
        _cache["nc"] = _build_nc()
    return _cache["nc"]


def kernel(**inputs):
    import time

    from concourse.bass_utils import run_bass_kernel_spmd

    nc = get_nc()
    in_maps = _shard(inputs)
    try:
        res = run_bass_kernel_spmd(nc, in_maps, core_ids=list(range(CORES)))
    except Exception:  # transient axon/NRT wedge: one retry
        time.sleep(20)
        res = run_bass_kernel_spmd(nc, in_maps, core_ids=list(range(CORES)))
    outs = [r["out"] for r in res.results]
    return np.concatenate(outs, axis=0)


# revision 50
# speedup vs baseline: 1.3127x; 1.0209x over previous
"""Trainium2 Bass kernel for the AnaphoricityScorer (coref pair scorer).

Reference computation (per mention row m, antecedent slot j):
    b = all_mentions[idx[m, j]]                       # gather, 1024-dim
    pair = [a_m, b, a_m * b, pw[m, j]]                # 3136-dim
    h = leaky_relu(pair @ W1 + b1)                    # 1024-dim
    score[m, j] = rough[m, j] + h @ W_out + b_out
    out[m] = [EPS, score[m, 0..49]]                   # [1024, 51]

Strategy (8 NeuronCores, data-parallel over the 1024-mention batch):
  * 128 mentions per core; all_mentions + FFNN weights replicated.
  * Decompose W1 by pair-feature block: W1 = [W1_a; W1_b; W1_ab; W1_pw].
      - a-term:  A1^T = (W1_a^T a^T) computed once per core (tiny).
      - b-term:  BM = all_mentions @ W1_b precomputed SHARDED across the 8
        cores (1024 rows each) + AllGather, then per-pair rows of BM are
        *gathered* instead of re-multiplied (saves 13.4 GFLOP/core).
      - ab-term: full matmul (a (x) b)^T @ ... -> irreducible 13.4 GFLOP/core,
        run in fp8e4m3 with MatmulPerfMode.DoubleRow (2 K-chunks of 128 per
        instruction, ~2x stream rate; measured 2419 cyc vs bf16 4096 per
        K=1024 x N=512 group). W1 blocks carry a x64 scale (keeps e4m3 out of
        subnormals); leaky-relu is positively homogeneous, so /64 baked into
        W_out cancels it. rel err 1.0e-2 vs 1.8e-3 in bf16 (gate 2e-2).
      - pw-term: K=64 bf16 matmul.
  * dma_gather(transpose=True) both gathers antecedent rows AND delivers them
    feature-major ([emb%128, emb//128, paircol]) - exactly the moving-operand
    layout the matmul needs. Pair columns are ordered g = j*128 + m so each
    128-column group shares one antecedent slot j across all 128 mentions.
  * Per 512-column block and h-chunk, the PE stream is [4x ab DoubleRow + pw
    + identity-matmul(ba)]: the b-term enters PSUM via a PE identity matmul
    whose rhs ba = bmT + A1^T is precomputed on the DVE one block AHEAD
    (bmT gather + DVE add prefetched), and layer-2 (N=1 matmuls vs W_out)
    is DEFERRED one block - so the in-order PE queue never waits on a
    same-block DVE/scalar/gather product (DVE adds into PSUM measured
    3.1us/block exposed in the fused form). leaky-relu on the ScalarEngine
    (Lrelu) -> bf16 h^T consumed by next block's layer-2.
  * bT gathers prefetched 4 blocks ahead (the first bmT gather stalls the
    FIFO gather queue until the AllGather lands).
  * ALL inputs are packed into ONE bf16 DRAM blob per core (f32/i16/fp8
    sections bitcast) - the axon per-execute dispatch cost scales with the
    input tensor count (~60-120us per input x device), so 17 inputs -> 1
    (measured -870us/exec head-to-head).
"""

import numpy as np
import ml_dtypes

CORES = 8
BATCH = 1024
B_LOC = BATCH // CORES          # 128 mentions per core
N_ANTS = 50
EMB = 1024
PW = 64
HID = 1024
N_MENT = 8192
G = B_LOC * N_ANTS              # 6400 pair columns per core
EPS = 1e-7

COLS_PER_BLK = 512              # pair columns per block (4 antecedent slots)
N_BLK = (G + COLS_PER_BLK - 1) // COLS_PER_BLK   # 13 (12 full + 1 half)

SIM_SAFE = False                # set True before get_nc() for CoreSim runs
import os as _os
DBG_NO_COLL = _os.environ.get("KDBG_NO_COLL", "0") == "1"   # skip AllGather (wrong results)
DBG_NBLK = int(_os.environ.get("KDBG_NBLK", "0"))           # limit main-loop blocks
DBG_NO_LRELU = _os.environ.get("KDBG_NO_LRELU", "0") == "1"
DBG_REPEAT = int(_os.environ.get("KDBG_REPEAT", "1"))  # repeat main loop (timing)
DBG_AG_REPEAT = int(_os.environ.get("KDBG_AG_REPEAT", "1"))  # repeat AllGather
DBG_BM_REPEAT = int(_os.environ.get("KDBG_BM_REPEAT", "1"))  # repeat BM compute
DBG_NO_BT = _os.environ.get("KDBG_NO_BT", "0") == "1"    # skip bT gather (wrong)
DBG_NO_BMT = _os.environ.get("KDBG_NO_BMT", "0") == "1"  # skip bmT gather+add (wrong)
DBG_NO_BA = _os.environ.get("KDBG_NO_BA", "0") == "1"    # keep gather, skip adds (wrong)
FP8 = _os.environ.get("KDBG_FP8", "1") == "1"  # ab-term matmuls in fp8e4m3
# Blocks whose ab+pw psum is staged to SBUF during the AllGather; their
# b-term is combined later on DVE/scalar while the normal loop runs on PE.
STAGE = int(_os.environ.get("KDBG_STAGE", "1"))
FP8_SCALE = 64.0  # keeps 64*W1ab out of the e4m3 subnormal range; lrelu is
# positively homogeneous so baking x64 into W1 and /64 into W_out cancels

_BF16 = ml_dtypes.bfloat16

_cache = {}


def _prod(s):
    p = 1
    for x in s:
        p *= x
    return p


def _align(x, a=256):
    return (x + a - 1) // a * a


# ---- blob layout: (name, shape, elem-kind). Kinds: bf16/i16 (1 blob elem),
# f32 (2 blob elems), fp8 (1/2 blob elem). Offsets in bf16 elems, 256-aligned.
def _sections():
    return [
        ("am",    [N_MENT, EMB],        "bf16"),
        ("ident", [128, 128],           "bf16"),
        ("w1a",   [128, 8, HID],        "bf16"),
        ("w1b",   [128, 8, HID],        "bf16"),
        ("w1ab",  [128, 8, HID],        "fp8" if FP8 else "bf16"),
        ("w1pw",  [PW, HID],            "bf16"),
        ("aT",    [128, 8, B_LOC],      "bf16"),
        ("aTr",   [128, 8, 512],        "bf16"),
        ("pwT",   [PW, N_ANTS, B_LOC],  "bf16"),
        ("wout",  [128, 8],             "bf16"),
        ("rough", [B_LOC, N_ANTS],      "f32"),
        ("b1T",   [128, 8],             "f32"),
        ("boutr", [128, 1],             "f32"),
        ("idx",   [128, G // 16],       "i16"),
        ("idxid", [128, B_LOC * 8 // 16], "i16"),
    ]


def _elems(shape, kind):
    n = _prod(shape)
    if kind == "f32":
        return n * 2
    if kind == "fp8":
        assert n % 2 == 0
        return n // 2
    return n


def _blob_offsets():
    offs, off = {}, 0
    for name, shape, kind in _sections():
        offs[name] = off
        off = _align(off + _elems(shape, kind))
    return offs, off


def _build_nc():
    from contextlib import ExitStack

    import concourse.bacc as bacc
    import concourse.mybir as mybir
    import concourse.tile as tile

    f32 = mybir.dt.float32
    bf16 = mybir.dt.bfloat16
    i16 = mybir.dt.int16
    fp8 = mybir.dt.float8e4
    AF = mybir.ActivationFunctionType
    ALU = mybir.AluOpType

    nc = bacc.Bacc("TRN2", target_bir_lowering=False, debug=False,
                   num_devices=CORES)

    offs, blob_len = _blob_offsets()
    blob_d = nc.dram_tensor("blob", [blob_len], bf16, kind="ExternalInput")
    out_d = nc.dram_tensor("out", [B_LOC, 1 + N_ANTS], f32, kind="ExternalOutput")

    _dt = {"bf16": bf16, "f32": f32, "i16": i16, "fp8": fp8}
    _kinds = {name: kind for name, _, kind in _sections()}
    _shapes = {name: shape for name, shape, _ in _sections()}

    def view(name):
        shape, kind = _shapes[name], _kinds[name]
        ap = blob_d[offs[name]:offs[name] + _elems(shape, kind)]
        if kind != "bf16":
            ap = ap.bitcast(_dt[kind])
        if len(shape) == 1:
            return ap
        dims = "abcde"[:len(shape)]
        kw = {d: s for d, s in zip(dims, shape)}
        return ap.rearrange(f"({' '.join(dims)}) -> {' '.join(dims)}", **kw)

    with tile.TileContext(nc) as tc, ExitStack() as ctx:
        const = ctx.enter_context(tc.tile_pool(name="const", bufs=1))
        dram = ctx.enter_context(tc.tile_pool(name="dram", bufs=1, space="DRAM"))
        gpool = ctx.enter_context(tc.tile_pool(name="gather", bufs=2))
        wpool = ctx.enter_context(tc.tile_pool(name="work", bufs=2))
        hpool = ctx.enter_context(tc.tile_pool(name="hsb", bufs=2))
        plp_cm = tc.tile_pool(name="prolog", bufs=1)
        plp = plp_cm.__enter__()
        pp = ctx.enter_context(tc.tile_pool(name="psum", bufs=5, space="PSUM"))
        fp = ctx.enter_context(tc.tile_pool(name="fpsum", bufs=3, space="PSUM"))

        def load(name, dt, eng=None, pool=None):
            shape = _shapes[name]
            t = (pool or const).tile(shape, dt, tag=name)
            (eng or nc.sync).dma_start(t[:], view(name))
            return t

        # load order matters: idx tiles + small consts first so block-0
        # gathers and matmuls can start while the big weights stream in.
        # Big weights go on the scalar HWDGE queue, small/critical on sync.
        idx_sb = load("idx", i16)
        idxid_sb = load("idxid", i16)
        ident = load("ident", bf16)
        w1b = load("w1b", bf16, pool=plp)
        aT = load("aT", bf16, eng=nc.scalar)
        aTr = load("aTr", bf16, eng=nc.scalar)
        wout = load("wout", bf16, eng=nc.scalar)
        b1T = load("b1T", f32, eng=nc.scalar)
        bout = load("boutr", f32, eng=nc.scalar)
        rough = load("rough", f32, eng=nc.scalar)
        w1ab = load("w1ab", fp8 if FP8 else bf16, eng=nc.scalar)
        pwT = load("pwT", bf16, eng=nc.scalar)
        w1pw = load("w1pw", bf16, eng=nc.scalar)
        w1a = load("w1a", bf16, eng=nc.scalar, pool=plp)

        am_ap = view("am")

        # ---- BM = all_mentions @ W1_b for my 1024-row shard, then AllGather.
        # amT: my shard of all_mentions, feature-major, via identity gather.
        # Split into 512-idx halves: one gather's descriptors must fit the
        # SWDGE ring carveout (1024-idx transpose gathers fault on HW).
        amT_h = [plp.tile([128, 8, 256], bf16, tag=f"amT{h}", name=f"amT{h}")
                 for h in range(4)]

        bm_shard = dram.tile([B_LOC * 8, HID], bf16)

        for r in range(DBG_BM_REPEAT):
            for h in range(4):
                nc.gpsimd.dma_gather(amT_h[h][:], am_ap,
                                     idxid_sb[:, h * 16:(h + 1) * 16],
                                     256, 256, EMB, transpose=True)

            for mi in range(8):
                for nh in range(2):
                    ps = pp.tile([128, 512], f32, tag="hp")
                    amT = amT_h[mi // 2]
                    mo = (mi % 2) * 128
                    for ce in range(8):
                        nc.tensor.matmul(
                            ps[:],
                            amT[:, ce, mo:mo + 128],
                            w1b[:, ce, nh * 512:(nh + 1) * 512],
                            start=(ce == 0), stop=(ce == 7))
                    # bf16 convert on the (idle) DVE, small 2-buf bounce tile
                    bmc = plp.tile([128, 512], bf16, tag="bmc", bufs=2)
                    nc.vector.tensor_scalar_mul(bmc[:], ps[:], 1.0)
                    nc.sync.dma_start(
                        bm_shard[mi * 128:(mi + 1) * 128,
                                 nh * 512:(nh + 1) * 512], bmc[:])

        for r in range(DBG_AG_REPEAT):
            bm_full = dram.tile([N_MENT, HID], bf16, addr_space="Shared",
                                name=f"bm_full{r}")
            if not DBG_NO_COLL:
                nc.gpsimd.collective_compute(
                    "AllGather", ALU.bypass,
                    replica_groups=[list(range(CORES))],
                    ins=[bm_shard.opt()],
                    outs=[bm_full.opt()],
                )
            else:
                nc.sync.dma_start(bm_full[0:1024, :], bm_shard[:])

        # ---- pw-term pass: h_pw[h, (j,m)] = W1pw^T @ pwT for ALL blocks,
        # computed in the AllGather window (PE idles there) and parked in
        # DRAM - removes the per-block K=64 pw matmul (1.76us/block) from
        # the steady-state PE stream; it rejoins via the ba DVE add.
        nblk0 = N_BLK if DBG_NBLK == 0 else max(DBG_NBLK, 0)
        hpw_d = dram.tile([nblk0, 128, 8 * COLS_PER_BLK], bf16)
        for B0 in range(nblk0):
            njp = min(4, N_ANTS - B0 * 4)
            ncolp = njp * 128
            for ch in range(8):
                ps = pp.tile([128, ncolp], f32, tag="hp")
                nc.tensor.matmul(ps[:], w1pw[:, ch * 128:(ch + 1) * 128],
                                 pwT[:, B0 * 4:B0 * 4 + njp, :],
                                 start=True, stop=True)
                pwc = plp.tile([128, ncolp], bf16, tag="pwc", bufs=2)
                nc.vector.tensor_scalar_mul(pwc[:], ps[:], 1.0)
                nc.sync.dma_start(
                    hpw_d[B0, :, ch * COLS_PER_BLK:ch * COLS_PER_BLK + ncolp],
                    pwc[:])

        # ---- A1^T = (a @ W1_a + b1)^T : [h%128, h//128, m], bf16,
        # replicated x4 along cols so the per-block add is one N=512 matmul.
        a1Tr = const.tile([128, 8, 512], bf16)
        for ch in range(8):
            ps = pp.tile([128, B_LOC], f32, tag="hp")
            for ce in range(8):
                nc.tensor.matmul(ps[:], w1a[:, ce, ch * 128:(ch + 1) * 128],
                                 aT[:, ce, :], start=(ce == 0), stop=(ce == 7))
            for j in range(4):
                nc.vector.tensor_scalar_add(a1Tr[:, ch, j * 128:(j + 1) * 128],
                                            ps[:], b1T[:, ch:ch + 1])

        plp_cm.__exit__(None, None, None)
        # staged-block tiles live only after the prologue pool closes -
        # opened here so the allocator can reuse plp's bytes
        spool = ctx.enter_context(tc.tile_pool(name="stage", bufs=1))

        # ---- main loop over pair-column blocks
        scores = const.tile([B_LOC, 1 + N_ANTS], f32)
        nc.vector.memset(scores[:, 0:1], EPS)

        nblk = N_BLK if DBG_NBLK == 0 else max(DBG_NBLK, 0)
        PREF = 4   # bT gathers issued this many blocks ahead of bmT (the
        # bmT gather stalls the FIFO gather queue until the AllGather lands)
        blocks = [b for _ in range(DBG_REPEAT) for b in range(nblk)]

        def block_geom(B):
            nj = min(4, N_ANTS - B * 4)
            return nj, nj * 128, B * (COLS_PER_BLK // 16)

        def issue_bT(B):
            nj, NCOL, i0 = block_geom(B)
            t = gpool.tile([128, 8, NCOL], bf16, tag="bT", name=f"bT{B}", bufs=4)
            if DBG_NO_BT:
                # timing variant: same bytes via plain contiguous HWDGE DMA
                st = B * 128 * 8 * NCOL
                nc.sync.dma_start(t[:], blob_d[st:st + 128 * 8 * NCOL]
                                  .rearrange("(p c n) -> p c n", p=128, c=8))
            else:
                nc.gpsimd.dma_gather(t[:], am_ap,
                                     idx_sb[:, i0:i0 + NCOL // 16],
                                     NCOL, NCOL, EMB, transpose=True)
            return t

        bT_q = {}
        for i, B in enumerate(blocks[:PREF]):
            bT_q[i] = issue_bT(B)

        # bmT gather -> ba = bmT + A1^T and abT = bT * aT are produced one
        # block AHEAD of PE consumption, so the PE's in-order queue never
        # waits on a same-block DVE/gather product.
        ab_q = {}
        no_b = DBG_NO_BMT or DBG_NO_BA
        # staged blocks run ab+pw during the AllGather, b-term combined later
        stage_n = 0 if no_b else min(STAGE, max(0, len(blocks) - 1))

        def issue_ba(bi2, tag="ba", bufs=None):
            B2 = blocks[bi2]
            nj2, NCOL2, i02 = block_geom(B2)
            bmT = gpool.tile([128, 8, NCOL2], bf16, tag="bmT")
            nc.gpsimd.dma_gather(bmT[:], bm_full[:],
                                 idx_sb[:, i02:i02 + NCOL2 // 16],
                                 NCOL2, NCOL2, HID, transpose=True)
            hpw = spool.tile([128, 8, NCOL2], bf16, tag="hpw", bufs=2)
            nc.scalar.dma_start(
                hpw[:], hpw_d[B2, :, :]
                .rearrange("p (c n) -> p c n", c=8)[:, :, 0:NCOL2])
            ba = wpool.tile([128, 8, NCOL2], bf16, tag=tag, bufs=bufs)
            nc.vector.tensor_add(ba[:], bmT[:], a1Tr[:, :, 0:NCOL2])
            nc.vector.tensor_add(ba[:], ba[:], hpw[:])
            return ba

        def issue_ab(bi2):
            B2 = blocks[bi2]
            nj2, NCOL2, i02 = block_geom(B2)
            abT = wpool.tile([128, 8, NCOL2], fp8 if FP8 else bf16, tag="abT")
            nc.vector.tensor_mul(abT[:], bT_q[bi2][:], aTr[:, :, 0:NCOL2])
            ba = None
            if not no_b and bi2 >= stage_n:
                ba = issue_ba(bi2)
            ab_q[bi2] = (abT, ba)

        issue_ab(0)

        # staged pipeline state: s_wait -> (gather+ba) -> s_ready ->
        # (DVE add + scalar lrelu) -> s_comb -> (PE layer-2 + scores)
        s_wait = list(range(stage_n))
        s_ready = []   # (bi, ba_tile)
        s_comb = []    # (bi, hT_tile)
        hpre = {}      # bi -> staged ab+pw SBUF tile

        # Per block: PE runs [4x ab DoubleRow + pw + identity-add(ba)] per
        # h-chunk; the b-term lands via the identity matmul (rhs = ba =
        # bmT + A1^T, precomputed on DVE with no dependence on this block's
        # psum), and layer-2 matmuls are DEFERRED one block so the in-order
        # PE queue never waits on this block's lrelu output.
        prev = None  # (hT, ffps, nj, j0) of previous block

        def issue_l2(pv, ch):
            hT_p, ffps_p, nj_p, _ = pv
            for j in range(nj_p):
                nc.tensor.matmul(ffps_p[:, j:j + 1],
                                 hT_p[:, ch, j * 128:(j + 1) * 128],
                                 wout[:, ch:ch + 1],
                                 start=(ch == 0 and j == 0),
                                 stop=(ch == 7 and j == nj_p - 1))

        def finish_scores(pv):
            _, ffps_p, nj_p, j0_p = pv
            # scores[:, 1+j0 : 1+j0+nj] = ffnn + b_out + rough
            nc.vector.scalar_tensor_tensor(scores[:, 1 + j0_p:1 + j0_p + nj_p],
                                           ffps_p[:], bout[:, 0:1],
                                           rough[:, j0_p:j0_p + nj_p],
                                           op0=ALU.add, op1=ALU.add)

        def staged_step():
            # one pipeline advance; each staged block gets >= one normal
            # iteration between its stages, so PE/DVE/scalar never wait on
            # a same-iteration product.
            if s_comb:
                s0, hT_s = s_comb.pop(0)
                nj0, NCOL0, _ = block_geom(blocks[s0])
                ffps_s = fp.tile([128, nj0], f32, tag="ff")
                for ch in range(8):
                    for j in range(nj0):
                        nc.tensor.matmul(ffps_s[:, j:j + 1],
                                         hT_s[:, ch, j * 128:(j + 1) * 128],
                                         wout[:, ch:ch + 1],
                                         start=(ch == 0 and j == 0),
                                         stop=(ch == 7 and j == nj0 - 1))
                finish_scores((None, ffps_s, nj0, blocks[s0] * 4))
            if s_ready:
                s1, ba_s = s_ready.pop(0)
                nj1, NCOL1, _ = block_geom(blocks[s1])
                t = spool.tile([128, 8, NCOL1], bf16, tag="ts", bufs=1)
                nc.vector.tensor_add(t[:], hpre.pop(s1)[:], ba_s[:])
                hT_s = spool.tile([128, 8, NCOL1], bf16, tag="hTs", bufs=2)
                nc.scalar.activation(hT_s[:], t[:], AF.Lrelu, alpha=0.01)
                s_comb.append((s1, hT_s))
            if s_wait:
                s2 = s_wait.pop(0)
                s_ready.append((s2, issue_ba(s2, tag="bas", bufs=1)))

        for bi, B in enumerate(blocks):
            nj, NCOL, i0 = block_geom(B)
            j0 = B * 4
            staged = bi < stage_n

            if bi + PREF < len(blocks):
                bT_q[bi + PREF] = issue_bT(blocks[bi + PREF])
            abT, ba = ab_q.pop(bi)
            bT_q.pop(bi)
            if bi + 1 < len(blocks):
                issue_ab(bi + 1)
            use_ba = ba is not None

            if not staged:
                ffps = fp.tile([128, nj], f32, tag="ff")
                hT = hpool.tile([128, 8, NCOL], bf16, tag="hT")
            else:
                hpre[bi] = spool.tile([128, 8, NCOL], bf16, tag=f"hpre{bi}",
                                      name=f"hpre{bi}", bufs=1)
            for ch in range(8):
                hp = pp.tile([128, NCOL], f32, tag="hp")
                if FP8:
                    # DoubleRow: 2 K-chunks of 128 per instruction, 2 rows
                    # per PE cell - halves both stream cycles and instr count
                    for cp in range(4):
                        nc.tensor.matmul(
                            hp[:],
                            w1ab[:, 2 * cp:2 * cp + 2, ch * 128:(ch + 1) * 128],
                            abT[:, 2 * cp:2 * cp + 2, :],
                            start=(cp == 0), stop=(cp == 3 and not use_ba),
                            perf_mode=mybir.MatmulPerfMode.DoubleRow)
                else:
                    for ce in range(8):
                        nc.tensor.matmul(hp[:],
                                         w1ab[:, ce, ch * 128:(ch + 1) * 128],
                                         abT[:, ce, :], start=(ce == 0),
                                         stop=(ce == 7 and not use_ba))
                if use_ba:
                    nc.tensor.matmul(hp[:], ident[:],
                                     ba[:, ch, :], start=False, stop=True)
                if not staged and prev is not None:
                    issue_l2(prev, ch)

                if staged:
                    # park ab+pw in SBUF; b-term combined post-AllGather
                    nc.scalar.activation(hpre[bi][:, ch, :], hp[:], AF.Copy)
                elif SIM_SAFE or DBG_NO_LRELU:  # CoreSim lacks Lrelu
                    lt = hpool.tile([128, NCOL], f32, tag="lt")
                    nc.vector.tensor_scalar_mul(lt[:], hp[:], 0.01)
                    nc.vector.tensor_max(hT[:, ch, :], hp[:], lt[:])
                else:
                    nc.scalar.activation(hT[:, ch, :], hp[:], AF.Lrelu, alpha=0.01)

            if not staged:
                if prev is not None:
                    finish_scores(prev)
                prev = (hT, ffps, nj, j0)
                staged_step()

        # epilogue: layer-2 + scores for the final normal block, then drain
        # the staged pipeline
        if prev is not None:
            for ch in range(8):
                issue_l2(prev, ch)
            finish_scores(prev)
        while s_wait or s_ready or s_comb:
            staged_step()

        nc.sync.dma_start(out_d[:], scores[:])

    nc.compile()
    return nc


def _shard(inputs):
    am = np.asarray(inputs["all_mentions"], np.float32)
    mb_ = np.asarray(inputs["mentions_batch"], np.float32)
    pw = np.asarray(inputs["pw_batch"], np.float32)
    rough = np.asarray(inputs["top_rough_scores_batch"], np.float32)
    W1 = np.asarray(inputs["W1"], np.float32)
    b1 = np.asarray(inputs["b1"], np.float32)
    Wout = np.asarray(inputs["W_out"], np.float32)
    bout = np.asarray(inputs["b_out"], np.float32)
    idx = np.asarray(inputs["top_indices_batch"])

    am_bf = am.astype(_BF16)
    S = FP8_SCALE if FP8 else 1.0

    def wblock(rows, dt=_BF16):  # [1024, 1024] -> [128, 8, 1024] (e%128, e//128, h)
        return np.ascontiguousarray(
            rows.reshape(8, 128, HID).transpose(1, 0, 2)).astype(dt)

    w1a = wblock(S * W1[0:1024])
    w1b = wblock(S * W1[1024:2048])
    w1ab = wblock(S * W1[2048:3072],
                  ml_dtypes.float8_e4m3fn if FP8 else _BF16)
    w1pw = np.ascontiguousarray(S * W1[3072:3136]).astype(_BF16)      # [64, 1024]
    wout = np.ascontiguousarray(
        Wout[:, 0].reshape(8, 128).T / S).astype(_BF16)
    b1T = np.ascontiguousarray(S * b1.reshape(8, 128).T).astype(np.float32)
    boutr = np.full((128, 1), bout[0], np.float32)

    def wrap_idx(flat, pad_cols):
        # [16, n/16] wrapped block, replicated across all 8 GPSIMD-core
        # partition groups (the odd Q7 core reads partitions 16-31).
        return np.tile(flat.reshape(pad_cols, 16).T, (8, 1))

    def as_u16(a):
        a = np.ascontiguousarray(a)
        if a.dtype.itemsize == 1:
            a = a.reshape(-1)
            assert a.size % 2 == 0
            return a.view(np.uint16)
        return a.view(np.uint16).reshape(-1)

    in_maps = []
    for c in range(CORES):
        sl = slice(c * B_LOC, (c + 1) * B_LOC)
        aT = np.ascontiguousarray(
            mb_[sl].T.reshape(8, 128, B_LOC).transpose(1, 0, 2)).astype(_BF16)
        aTr = np.ascontiguousarray(np.tile(aT, (1, 1, 4)))
        pwT = np.ascontiguousarray(pw[sl].transpose(2, 1, 0)).astype(_BF16)
        idx_flat = np.ascontiguousarray(idx[sl].astype(np.int16).T).reshape(G)
        idxid_flat = np.arange(c * B_LOC * 8, (c + 1) * B_LOC * 8, dtype=np.int16)
        sections = {
            "am": am_bf,
            "ident": np.eye(128, dtype=_BF16),
            "aT": aT,
            "aTr": aTr,
            "pwT": pwT,
            "rough": np.ascontiguousarray(rough[sl]),
            "w1a": w1a, "w1b": w1b, "w1ab": w1ab, "w1pw": w1pw,
            "wout": wout, "b1T": b1T, "boutr": boutr,
            "idx": wrap_idx(idx_flat, G // 16),
            "idxid": wrap_idx(idxid_flat, B_LOC * 8 // 16),
        }
        offs, blob_len = _blob_offsets()
        blob = np.zeros(blob_len, np.uint16)
        for name, shape, kind in _sections():
            u = as_u16(sections[name])
            blob[offs[name]:offs[name] + u.size] = u
        in_maps.append({"blob": blob.view(_BF16)})
    return in_maps


def get_nc():
    if "nc" not in _cache:
        _cache["nc"] = _build_nc()
    return _cache["nc"]


def kernel(**inputs):
    import time

    from concourse.bass_utils import run_bass_kernel_spmd

    nc = get_nc()
    in_maps = _shard(inputs)
    try:
        res = run_bass_kernel_spmd(nc, in_maps, core_ids=list(range(CORES)))
    except Exception:  # transient axon/NRT wedge: one retry
        time.sleep(20)
        res = run_bass_kernel_spmd(nc, in_maps, core_ids=list(range(CORES)))
    outs = [r["out"] for r in res.results]
    return np.concatenate(outs, axis=0)
